# revision 29
# baseline (speedup 1.0000x reference)
"""DyRep classifier Bass kernel for 8 Trainium2 NeuronCores.

Strategy (self-contained; shapes hardcoded for the target problem):
  - The output depends only on per-label-node rows of (memory_buf,
    node_state[post-event], last_seen[post-event], node_features).
  - Host packs the four per-node tables into one 768-byte row per node
    (vs 1024B for the bf16-only layout):
      block0 (256B): interleaved fp8 pairs (mem[k]*8, state[k]*8) — only
        ever read by DVE element-wise ops (strided byte APs), never as a
        matmul operand (HW mixed-dtype matmul bf16xfp8 produces garbage).
      block1 (256B): feat[0:128] bf16 — matmul rhs.
      block2 (256B): dec_pre=exp(relu(decay)*last_seen) bf16 at unit 0
        (partition 0 => GpSimd partition_broadcast consumes it directly;
        no exp on device: dec = dec_pre * exp(-rd*T) via an ACT scale),
        feat[128:172] bf16 at units 1..44 (matmul rhs with a zero row 0).
    Sharded row-wise into 16 chunks of 31250 rows (int16-indexable);
    each of the 8 cores owns 2 chunks.
  - Host routes each unique label node to its owner (core, chunk) and
    splits into "untouched" / "touched" (touched = node hit by the
    event batch, needs the GRU update applied on the fly).
  - Device per core: dma_gather(transpose=True) delivers rows
    feature-major. The decayed-state scaling is fused on DVE
    (sstate = state8 * dec_bcast; t1 = sstate + mem8, both reading the
    fp8 pairs directly), W1 pre-divided by 8 compensates the x8 table
    scale. h1 psum accumulates 3 matmuls (featA, featB, W1@t1), ACT
    relu with fused bias, then W2 col-tiled so both supertiles of a
    gather land in ONE [114, 448] psum bank (s0 -> partitions 0:50,
    s1 -> 64:114), one DVE bias-copy, bf16 half-width output.
  - Host unpermutes the per-core outputs back to label order.
"""

import functools
import os
import numpy as np
import ml_dtypes

import concourse.bass as bass
import concourse.mybir as mybir
import concourse.tile as tile
from concourse import bacc
from concourse.bass_utils import run_bass_kernel_spmd

BF16 = ml_dtypes.bfloat16
FP8 = ml_dtypes.float8_e4m3

# Problem dims (fixed by the task)
N = 500000
H = 128
F = 172
C = 50
B = 200000

NCORES = 8
NCHUNK = 16                  # index chunks (int16 addressing limit)
CH = N // NCHUNK             # 31250 rows per chunk
ROW = 384                    # bf16-unit elements per packed row (768 bytes)
G = int(os.environ.get("K_G", "896"))   # occurrences per gather
# (transpose dma_gather num_idxs is HW-capped at 1016: 896 largest %128)
S = G // 2                   # supertile: occurrences per compute slice
FSCALE = 8.0                 # host scales mem/state by 8 into fp8 range
OC = 114                     # packed output partitions (2 x C, col-tiled)

f32 = mybir.dt.float32
bf16 = mybir.dt.bfloat16
fp8 = mybir.dt.float8e4
i16 = mybir.dt.int16
AF = mybir.ActivationFunctionType
OP = mybir.AluOpType

USE_PBCAST = os.environ.get("K_PBCAST", "1") == "1"  # dec broadcast on GpSimd


def _wrap_idxs(idx: np.ndarray) -> np.ndarray:
    """Wrap a flat int16 index list into the [128, n/16] SWDGE layout:
    element j at [j%16, j//16], replicated into all 8 16-partition groups."""
    n = idx.shape[0]
    assert n % 16 == 0
    cols = n // 16
    t = np.empty((128, cols), dtype=np.int16)
    blk = idx.reshape(cols, 16).T  # [16, cols]
    for k in range(8):
        t[k * 16:(k + 1) * 16, :] = blk
    return t


def build_program(u_pad: int, t_pad: int, ch: int = CH):
    """Build + compile the SPMD Bass program. Cached by padded sizes."""
    nc = bacc.Bacc("TRN2", target_bir_lowering=False, debug=False,
                   num_devices=NCORES)

    dt_in = {}

    def din(name, shape, dt):
        dt_in[name] = nc.dram_tensor(name, shape, dt, kind="ExternalInput").ap()
        return dt_in[name]

    tab_a = din("tab_a", (ch, ROW), bf16)
    tab_b = din("tab_b", (ch, ROW), bf16)
    uidx_a = din("uidx_a", (128, u_pad // 16), i16)
    uidx_b = din("uidx_b", (128, u_pad // 16), i16)
    tidx_a = din("tidx_a", (128, t_pad // 16), i16)
    tidx_b = din("tidx_b", (128, t_pad // 16), i16)

    wfa = din("wfa", (128, 128), bf16)       # (W1@W_feat).T[0:128]
    wfb = din("wfb", (45, 128), bf16)        # row0 zero, rows1..44 WcT[128:]
    w1t = din("w1t", (128, 128), bf16)       # W1.T / 8
    w2t = din("w2t", (128, 64), bf16)        # [W2.T | zeros] (64 cols)
    whhrt = din("whhrt", (128, 128), bf16)   # W_hh[0:128].T
    whhzt = din("whhzt", (128, 128), bf16)   # W_hh[128:256].T
    whhnt = din("whhnt", (128, 128), bf16)   # W_hh[256:384].T
    ones1 = din("ones1", (1, 128), bf16)
    b1p = din("b1p", (128, 1), f32)          # b1 + W1 @ b_feat
    b2v2 = din("b2v2", (OC, 1), f32)         # [b2; 0*14; b2]
    c_r = din("c_r", (128, 1), f32)          # gi_r + b_hh_r
    c_z = din("c_z", (128, 1), f32)          # gi_z + b_hh_z
    gin = din("gin", (128, 1), f32)          # gi_n
    bhn = din("bhn", (128, 1), f32)          # b_hh_n
    dect8 = din("dect8", (128, 1), f32)      # 8*exp(-relu(decay)*(T-t0))
    dtc = din("dtc", (128, 1), f32)          # exp(-relu(decay)*T)

    tch = (u_pad + t_pad)                    # out columns per core (half)
    out = nc.dram_tensor("out", (OC, tch), bf16, kind="ExternalOutput").ap()

    class W:
        pass

    with tile.TileContext(nc) as tc:
        with tc.tile_pool(name="wpool", bufs=1) as wp:
            for name in ("tidx_a", "tidx_b", "uidx_a", "uidx_b",
                         "wfa", "wfb", "w1t", "w2t", "whhrt", "whhzt",
                         "whhnt", "ones1", "b1p", "b2v2", "c_r", "c_z", "gin",
                         "bhn", "dect8", "dtc"):
                ap = dt_in[name]
                t = wp.tile(list(ap.shape), ap.dtype, tag=name)
                nc.sync.dma_start(t[:], ap[:])
                setattr(W, name, t)

            def supertile(sb, ps, ps2, p_o, X, s, touched):
                """One 448-occurrence compute slice; logits into
                p_o[0:50] (s=0) or p_o[64:114] (s=1, col-tiled)."""
                sl = bass.ds(s * S, S)
                X8 = X[:].bitcast(fp8)
                mem8 = X8[:, 0, bass.ds(2 * s * S, S, 2)]
                st8 = X8[:, 0, bass.ds(2 * s * S + 1, S, 2)]
                fAT = X[:, 1, sl]
                fBT = X[0:45, 2, sl]
                dpT = X[0:1, 2, sl]

                if not touched:
                    # dec = dec_pre(row) * exp(-rd*T) (ACT scale), then
                    # broadcast partition 0 -> 128 and scale the fp8 state.
                    dec_b = sb.tile([1, S], bf16, tag="dec_b")
                    nc.scalar.activation(dec_b[:], dpT, AF.Identity,
                                         scale=W.dtc[0:1, :])
                    sstate = sb.tile([128, S], bf16, tag="sstate")
                    if USE_PBCAST:
                        dec_sb = sb.tile([128, S], bf16, tag="dec_sb")
                        nc.gpsimd.partition_broadcast(dec_sb[:], dec_b[0:1, :])
                        nc.vector.tensor_tensor(out=sstate[:], in0=st8,
                                                in1=dec_sb[:], op=OP.mult)
                    else:
                        p_dec = ps.tile([128, S], f32, tag="dec")
                        nc.tensor.matmul(p_dec[:], lhsT=W.ones1[:],
                                         rhs=dec_b[:], start=True, stop=True)
                        nc.vector.tensor_tensor(out=sstate[:], in0=st8,
                                                in1=p_dec[:], op=OP.mult)
                else:
                    stc = sb.tile([128, S], bf16, tag="stc")
                    nc.vector.tensor_scalar_mul(stc[:], st8, 1.0 / FSCALE)
                    p_r = ps.tile([128, S], f32, tag="gr")
                    nc.tensor.matmul(p_r[:], lhsT=W.whhrt[:], rhs=stc[:],
                                     start=True, stop=True)
                    p_z = ps.tile([128, S], f32, tag="gz")
                    nc.tensor.matmul(p_z[:], lhsT=W.whhzt[:], rhs=stc[:],
                                     start=True, stop=True)
                    p_n = ps.tile([128, S], f32, tag="gn")
                    nc.tensor.matmul(p_n[:], lhsT=W.whhnt[:], rhs=stc[:],
                                     start=True, stop=True)
                    r = sb.tile([128, S], f32, tag="r")
                    nc.scalar.activation(r[:], p_r[:], AF.Sigmoid, bias=W.c_r[:])
                    z = sb.tile([128, S], f32, tag="z")
                    nc.scalar.activation(z[:], p_z[:], AF.Sigmoid, bias=W.c_z[:])
                    hn = sb.tile([128, S], f32, tag="hn")
                    nc.scalar.activation(hn[:], p_n[:], AF.Identity, bias=W.bhn[:])
                    rn = sb.tile([128, S], f32, tag="rn")
                    nc.vector.tensor_tensor(out=rn[:], in0=r[:], in1=hn[:],
                                            op=OP.mult)
                    n = sb.tile([128, S], f32, tag="n")
                    nc.scalar.activation(n[:], rn[:], AF.Tanh, bias=W.gin[:])
                    d = sb.tile([128, S], f32, tag="d")
                    nc.vector.tensor_tensor(out=d[:], in0=stc[:], in1=n[:],
                                            op=OP.subtract)
                    zd = sb.tile([128, S], f32, tag="zd")
                    nc.vector.tensor_tensor(out=zd[:], in0=z[:], in1=d[:],
                                            op=OP.mult)
                    ns = sb.tile([128, S], f32, tag="ns")
                    nc.vector.tensor_tensor(out=ns[:], in0=n[:], in1=zd[:],
                                            op=OP.add)
                    # x8 to match the table scale folded into W1
                    sstate = sb.tile([128, S], bf16, tag="sstate")
                    nc.vector.tensor_scalar_mul(sstate[:], ns[:],
                                                W.dect8[:, 0:1])

                t1 = sb.tile([128, S], bf16, tag="t1")
                nc.vector.tensor_tensor(out=t1[:], in0=sstate[:], in1=mem8,
                                        op=OP.add)
                p_h1 = ps2.tile([128, S], f32, tag="h1")
                nc.tensor.matmul(p_h1[:], lhsT=W.wfa[:], rhs=fAT,
                                 start=True, stop=False)
                nc.tensor.matmul(p_h1[:], lhsT=W.wfb[:], rhs=fBT,
                                 start=False, stop=False)
                nc.tensor.matmul(p_h1[:], lhsT=W.w1t[:], rhs=t1[:],
                                 start=False, stop=True)
                h1 = sb.tile([128, S], bf16, tag="h1s")
                nc.scalar.activation(h1[:], p_h1[:], AF.Relu, bias=W.b1p[:])
                o_sl = p_o[0:64, :] if s == 0 else p_o[64:128, :]
                nc.tensor.matmul(o_sl, lhsT=W.w2t[:], rhs=h1[:],
                                 start=True, stop=True)

            def stream(gp, sb, ps, ps2, pso, table_ap, idx_tile, n_occ,
                       col0h, touched):
                """Process one (chunk, touched?) stream of n_occ occurrences
                in gathers of G; outputs to out[:, col0h : col0h+n_occ/2]."""
                n_g = n_occ // G
                for g in range(n_g):
                    X = gp.tile([128, 3, G], bf16, tag=f"gath{touched}")
                    nc.gpsimd.dma_gather(
                        out_ap=X[:],
                        in_ap=table_ap[:],
                        idxs_ap=idx_tile[:, bass.ds(g * G // 16, G // 16)],
                        num_idxs=G,
                        num_idxs_reg=G,
                        elem_size=ROW,
                        transpose=True,
                    )
                    p_o = pso.tile([128, S], f32, tag=f"po{touched}")
                    for s in range(2):
                        supertile(sb, ps, ps2, p_o, X, s, touched)
                    osb = gp.tile([OC, S], bf16, tag=f"osb{touched}")
                    nc.vector.tensor_scalar_add(osb[:], p_o[0:OC, :],
                                                W.b2v2[:, 0:1])
                    nc.sync.dma_start(
                        out[:, bass.ds(col0h + g * S, S)], osb[:])

            # One deep gather pool shared by both phases so the Pool engine
            # streams descriptor generation continuously across the
            # touched->untouched transition. Touched first: its serial GRU
            # chain drains while untouched gathers already issue.
            with tc.tile_pool(name="gp", bufs=8) as gp:
                with tc.tile_pool(name="sbt", bufs=2) as sb, \
                     tc.tile_pool(name="pst", bufs=1, space="PSUM") as ps, \
                     tc.tile_pool(name="pst2", bufs=1, space="PSUM") as ps2, \
                     tc.tile_pool(name="psot", bufs=1, space="PSUM") as pso:
                    stream(gp, sb, ps, ps2, pso, tab_a, W.tidx_a, t_pad,
                           u_pad // 2, True)
                    stream(gp, sb, ps, ps2, pso, tab_b, W.tidx_b, t_pad,
                           u_pad + t_pad // 2, True)
                with tc.tile_pool(name="sbu", bufs=4) as sb, \
                     tc.tile_pool(name="psu", bufs=1, space="PSUM") as ps, \
                     tc.tile_pool(name="psu2", bufs=3, space="PSUM") as ps2, \
                     tc.tile_pool(name="psou", bufs=3, space="PSUM") as pso:
                    stream(gp, sb, ps, ps2, pso, tab_a, W.uidx_a, u_pad,
                           0, False)
                    stream(gp, sb, ps, ps2, pso, tab_b, W.uidx_b, u_pad,
                           (u_pad + t_pad) // 2, False)

    nc.compile()
    return nc


@functools.lru_cache(maxsize=4)
def _cached_program(u_pad, t_pad, ch):
    return build_program(u_pad, t_pad, ch)


def _round_up(x, m):
    return ((x + m - 1) // m) * m


def _prepare(label_nodes, src, dst, t, msg, memory_buf, node_state, last_seen,
             node_features, decay, W_msg, b_msg, W_ih, W_hh, b_ih, b_hh,
             W_feat, b_feat, W1, b1, W2, b2, current_time):
    """Host-side routing/packing. Returns (in_maps, meta)."""
    label_nodes = np.asarray(label_nodes)

    # ---- host: event-level scalars (O(1) work) ----
    t0 = float(np.asarray(t)[0])
    T = float(current_time)
    rdecay = max(float(decay), 0.0)
    event_msg = msg[0].astype(np.float64) @ W_msg.T.astype(np.float64) + b_msg
    gi = event_msg @ W_ih.T.astype(np.float64) + b_ih  # [3H], includes b_ih
    gi = gi.astype(np.float32)
    dec_t = np.float32(np.exp(-rdecay * (T - t0)))

    # ---- host: routing (dedup to unique label nodes) ----
    touched_nodes = np.unique(np.concatenate([src, dst]))
    uniq_vals, inv = np.unique(label_nodes, return_inverse=True)
    is_t = np.isin(uniq_vals, touched_nodes)
    chunk_id = uniq_vals // CH            # 0..15
    local = (uniq_vals % CH).astype(np.int16)

    key = chunk_id.astype(np.int64) * 2 + is_t
    order = np.argsort(key, kind="stable")
    counts = np.bincount(key, minlength=NCHUNK * 2)
    u_counts = counts[0::2]
    t_counts = counts[1::2]
    u_pad = max(_round_up(int(u_counts.max()), G), G)
    t_pad = max(_round_up(int(t_counts.max()), G), G)

    starts = np.zeros(NCHUNK * 2 + 1, dtype=np.int64)
    np.cumsum(counts, out=starts[1:])

    # ---- host: packed 768B-row table ----
    tab8 = np.zeros((N, 2 * ROW), dtype=np.uint8)
    m8 = (memory_buf * FSCALE).astype(FP8).view(np.uint8)
    s8 = (node_state * FSCALE).astype(FP8).view(np.uint8)
    tab8[:, 0:256:2] = m8
    tab8[:, 1:256:2] = s8
    tab8[:, 256:512] = node_features[:, 0:128].astype(BF16).view(np.uint8)
    blk = np.zeros((N, 256), dtype=np.uint8)
    dec_pre = np.exp(rdecay * last_seen.astype(np.float64)).astype(BF16)
    blk[:, 0:2] = dec_pre.reshape(-1, 1).view(np.uint8)
    blk[:, 2:90] = node_features[:, 128:172].astype(BF16).view(np.uint8)
    tab8[:, 512:768] = blk
    tab = tab8.view(BF16)                 # [N, 384] bf16-typed bytes

    # ---- host: weights / aux ----
    def bfc(x):
        return np.ascontiguousarray(x, dtype=BF16)

    def f32c(x):
        return np.ascontiguousarray(x, dtype=np.float32).reshape(-1, 1)

    WcT = (W1 @ W_feat).T  # [F, H] — W_feat folded through W1
    wfb = np.concatenate([np.zeros((1, H), np.float32), WcT[128:172]], axis=0)
    b2v2 = np.zeros(OC, np.float32)
    b2v2[0:C] = b2
    b2v2[64:64 + C] = b2
    aux = {
        "wfa": bfc(WcT[0:128]),
        "wfb": bfc(wfb),
        "w1t": bfc(W1.T / FSCALE),
        "w2t": bfc(np.concatenate(
            [W2.T, np.zeros((H, 64 - C), np.float32)], axis=1)),
        "whhrt": bfc(W_hh[0:128].T),
        "whhzt": bfc(W_hh[128:256].T),
        "whhnt": bfc(W_hh[256:384].T),
        "ones1": np.ones((1, 128), dtype=BF16),
        "b1p": f32c(b1 + W1 @ b_feat),
        "b2v2": f32c(b2v2),
        "c_r": f32c(gi[0:128] + b_hh[0:128]),
        "c_z": f32c(gi[128:256] + b_hh[128:256]),
        "gin": f32c(gi[256:384]),
        "bhn": f32c(b_hh[256:384]),
        "dect8": np.full((128, 1), dec_t * FSCALE, dtype=np.float32),
        "dtc": np.full((128, 1), np.exp(-rdecay * T), dtype=np.float32),
    }

    # ---- host: per-core input maps ----
    in_maps = []
    group_uids = {}  # (chunk, touched) -> unique-label ids in device order
    for ci in range(NCHUNK):
        for tf in (0, 1):
            k = ci * 2 + tf
            group_uids[(ci, tf)] = order[starts[k]:starts[k + 1]]

    def idx_input(ci, tf, pad):
        uids = group_uids[(ci, tf)]
        li = np.zeros(pad, dtype=np.int16)
        li[:uids.shape[0]] = local[uids]
        return _wrap_idxs(li)

    for core in range(NCORES):
        ca, cb = 2 * core, 2 * core + 1
        im = dict(aux)
        im["tab_a"] = tab[ca * CH:(ca + 1) * CH]
        im["tab_b"] = tab[cb * CH:(cb + 1) * CH]
        im["uidx_a"] = idx_input(ca, 0, u_pad)
        im["uidx_b"] = idx_input(cb, 0, u_pad)
        im["tidx_a"] = idx_input(ca, 1, t_pad)
        im["tidx_b"] = idx_input(cb, 1, t_pad)
        in_maps.append(im)

    # column (within a core's virtual [C, totcol] output) of each unique
    # label node; _finish first unpacks the [114, totcol/2] device layout.
    totcol = 2 * (u_pad + t_pad)
    col_of_uniq = np.empty(uniq_vals.shape[0], dtype=np.int64)
    for ci in range(NCHUNK):
        core = ci // 2
        for tf in (0, 1):
            uids = group_uids[(ci, tf)]
            if (ci % 2) == 0:
                c0 = 0 if tf == 0 else u_pad
            else:
                c0 = (u_pad + t_pad) if tf == 0 else (2 * u_pad + t_pad)
            col_of_uniq[uids] = core * totcol + c0 + np.arange(uids.shape[0])

    meta = {"u_pad": u_pad, "t_pad": t_pad, "col_of_uniq": col_of_uniq,
            "inv": inv, "nb": label_nodes.shape[0]}
    return in_maps, meta


def _finish(core_outs, meta):
    """Unpack [114, tch] col-tiled outputs and map back to label order."""
    outs50 = []
    for o in core_outs:
        o = np.asarray(o, dtype=np.float32)
        nh = o.shape[1]
        arr = o.reshape(OC, nh // S, S)
        st = np.stack([arr[0:C], arr[64:64 + C]], axis=2)  # [C, nh/S, 2, S]
        outs50.append(st.reshape(C, 2 * nh))
    combined = np.concatenate(outs50, axis=1)  # [C, NCORES*totcol]
    return np.ascontiguousarray(
        combined[:, meta["col_of_uniq"][meta["inv"]]].T)


def kernel(**inputs):
    inputs = {k: np.asarray(v) for k, v in inputs.items()}
    in_maps, meta = _prepare(**inputs)
    nc = _cached_program(meta["u_pad"], meta["t_pad"], CH)
    res = run_bass_kernel_spmd(nc, in_maps, core_ids=list(range(NCORES)))
    return _finish([r["out"] for r in res.results], meta)


# revision 30
# speedup vs baseline: 1.1679x; 1.1679x over previous
"""DyRep classifier Bass kernel for 8 Trainium2 NeuronCores.

Strategy (self-contained; shapes hardcoded for the target problem):
  - The output depends only on per-label-node rows of (memory_buf,
    node_state[post-event], last_seen[post-event], node_features).
  - Host packs the four per-node tables into one 768-byte row per node
    (vs 1024B for the bf16-only layout):
      block0 (256B): memory_buf bf16 — DVE operand.
      block1 (256B): node_state bf16 — GRU matmul rhs / DVE operand.
      block2 (256B): dec_pre=exp(relu(decay)*last_seen) bf16 at u16-unit
        0, feat*4 as fp8 pairs on units 1..127 (even bytes feat[1:128],
        odd bytes feat[128:172] on units 1..44 and feat[0] on unit 45).
    dec_pre is in [1,1.65) so BOTH its bytes decode as finite fp8 — the
    feat matmuls span it with zero weight rows without NaN risk. The
    feat matmuls run fp8 x fp8 (weights also *4; HW runs mixed
    bf16 x fp8 matmuls incorrectly, and fp8 weights at natural scale
    would be subnormal). The combined x16 is undone for free by the
    ACT scale on the h1 relu; W1 is pre-multiplied by 16 to match.
    No exp on device: dec = dec_pre * exp(-rd*T) via an ACT scale on a
    [1,S] slice, broadcast across partitions by a K=1 ones-matmul.
    Sharded row-wise into 16 chunks of 31250 rows (int16-indexable);
    each of the 8 cores owns 2 chunks.
  - Host routes each unique label node to its owner (core, chunk) and
    splits into "untouched" / "touched" (touched = node hit by the
    event batch, needs the GRU update applied on the fly).
  - Device per core: dma_gather(transpose=True) delivers rows
    feature-major. h1 psum accumulates 3 matmuls (featA fp8, featB fp8,
    W1@(mem + state*dec)), ACT relu with fused bias+scale, then W2
    (zero-padded to 64 rows) col-tiled so both supertiles of a gather
    land in ONE [128, 448] psum bank (s0 -> partitions 0:64,
    s1 -> 64:128), one DVE bias-copy, bf16 half-width output.
  - Host unpermutes the per-core outputs back to label order.
"""

import functools
import os
import numpy as np
import ml_dtypes

import concourse.bass as bass
import concourse.mybir as mybir
import concourse.tile as tile
from concourse import bacc
from concourse.bass_utils import run_bass_kernel_spmd

BF16 = ml_dtypes.bfloat16
FP8 = ml_dtypes.float8_e4m3

# Problem dims (fixed by the task)
N = 500000
H = 128
F = 172
C = 50
B = 200000

NCORES = 8
NCHUNK = 16                  # index chunks (int16 addressing limit)
CH = N // NCHUNK             # 31250 rows per chunk
ROW = 384                    # bf16-unit elements per packed row (768 bytes)
G = int(os.environ.get("K_G", "896"))   # occurrences per gather
# (transpose dma_gather num_idxs is HW-capped at 1016: 896 largest %128)
S = G // 2                   # supertile: occurrences per compute slice
FSC = 4.0                    # fp8 scale for feat AND its weights (x16 total)
OC = 114                     # packed output partitions (2 x C, col-tiled)

f32 = mybir.dt.float32
bf16 = mybir.dt.bfloat16
fp8 = mybir.dt.float8e4
i16 = mybir.dt.int16
AF = mybir.ActivationFunctionType
OP = mybir.AluOpType


def _wrap_idxs(idx: np.ndarray) -> np.ndarray:
    """Wrap a flat int16 index list into the [128, n/16] SWDGE layout:
    element j at [j%16, j//16], replicated into all 8 16-partition groups."""
    n = idx.shape[0]
    assert n % 16 == 0
    cols = n // 16
    t = np.empty((128, cols), dtype=np.int16)
    blk = idx.reshape(cols, 16).T  # [16, cols]
    for k in range(8):
        t[k * 16:(k + 1) * 16, :] = blk
    return t


def build_program(u_pad: int, t_pad: int, ch: int = CH):
    """Build + compile the SPMD Bass program. Cached by padded sizes."""
    nc = bacc.Bacc("TRN2", target_bir_lowering=False, debug=False,
                   num_devices=NCORES)

    dt_in = {}

    def din(name, shape, dt):
        dt_in[name] = nc.dram_tensor(name, shape, dt, kind="ExternalInput").ap()
        return dt_in[name]

    tab_a = din("tab_a", (ch, ROW), bf16)
    tab_b = din("tab_b", (ch, ROW), bf16)
    uidx_a = din("uidx_a", (128, u_pad // 16), i16)
    uidx_b = din("uidx_b", (128, u_pad // 16), i16)
    tidx_a = din("tidx_a", (128, t_pad // 16), i16)
    tidx_b = din("tidx_b", (128, t_pad // 16), i16)

    wfa = din("wfa", (128, 128), fp8)        # row0 0, rows1..127 WcT[1:128]*4
    wfb = din("wfb", (46, 128), fp8)         # row0 0, 1..44 WcT[128:]*4,
    #                                          row45 WcT[0]*4
    w1t = din("w1t", (128, 128), bf16)       # W1.T * 16
    w2t = din("w2t", (128, 64), bf16)        # [W2.T | zeros] (64 cols)
    whhrt = din("whhrt", (128, 128), bf16)   # W_hh[0:128].T
    whhzt = din("whhzt", (128, 128), bf16)   # W_hh[128:256].T
    whhnt = din("whhnt", (128, 128), bf16)   # W_hh[256:384].T
    ones1 = din("ones1", (1, 128), bf16)
    b1p = din("b1p", (128, 1), f32)          # b1 + W1 @ b_feat
    b2v2 = din("b2v2", (OC, 1), f32)         # [b2; 0*14; b2]
    c_r = din("c_r", (128, 1), f32)          # gi_r + b_hh_r
    c_z = din("c_z", (128, 1), f32)          # gi_z + b_hh_z
    gin = din("gin", (128, 1), f32)          # gi_n
    bhn = din("bhn", (128, 1), f32)          # b_hh_n
    dect = din("dect", (128, 1), f32)        # exp(-relu(decay)*(T-t0))
    dtc = din("dtc", (128, 1), f32)          # exp(-relu(decay)*T)

    tch = (u_pad + t_pad)                    # out columns per core (half)
    out = nc.dram_tensor("out", (OC, tch), bf16, kind="ExternalOutput").ap()

    class W:
        pass

    with tile.TileContext(nc) as tc:
        with tc.tile_pool(name="wpool", bufs=1) as wp:
            for name in ("tidx_a", "tidx_b", "uidx_a", "uidx_b",
                         "wfa", "wfb", "w1t", "w2t", "whhrt", "whhzt",
                         "whhnt", "ones1", "b1p", "b2v2", "c_r", "c_z", "gin",
                         "bhn", "dect", "dtc"):
                ap = dt_in[name]
                t = wp.tile(list(ap.shape), ap.dtype, tag=name)
                nc.sync.dma_start(t[:], ap[:])
                setattr(W, name, t)

            def supertile(sb, ps, ps2, p_o, X, s, touched):
                """One S-occurrence compute slice; logits into
                p_o[0:64] (s=0) or p_o[64:128] (s=1, col-tiled)."""
                sl = bass.ds(s * S, S)
                X8 = X[:].bitcast(fp8)
                memT = X[:, 0, sl]
                stT = X[:, 1, sl]
                fA = X8[:, 2, bass.ds(2 * s * S, S, 2)]
                fB = X8[0:46, 2, bass.ds(2 * s * S + 1, S, 2)]
                dpT = X[0:1, 2, sl]

                if not touched:
                    # dec = dec_pre(row) * exp(-rd*T) (ACT scale on [1,S]),
                    # partition-broadcast via K=1 ones-matmul, then scale
                    # the state on DVE reading the psum directly.
                    dec_b = sb.tile([1, S], bf16, tag="dec_b")
                    nc.scalar.activation(dec_b[:], dpT, AF.Identity,
                                         scale=W.dtc[0:1, :])
                    p_dec = ps.tile([128, S], f32, tag="dec")
                    nc.tensor.matmul(p_dec[:], lhsT=W.ones1[:],
                                     rhs=dec_b[:], start=True, stop=True)
                    sstate = sb.tile([128, S], bf16, tag="sstate")
                    nc.vector.tensor_tensor(out=sstate[:], in0=stT,
                                            in1=p_dec[:], op=OP.mult)
                else:
                    p_r = ps.tile([128, S], f32, tag="gr")
                    nc.tensor.matmul(p_r[:], lhsT=W.whhrt[:], rhs=stT,
                                     start=True, stop=True)
                    p_z = ps.tile([128, S], f32, tag="gz")
                    nc.tensor.matmul(p_z[:], lhsT=W.whhzt[:], rhs=stT,
                                     start=True, stop=True)
                    p_n = ps.tile([128, S], f32, tag="gn")
                    nc.tensor.matmul(p_n[:], lhsT=W.whhnt[:], rhs=stT,
                                     start=True, stop=True)
                    r = sb.tile([128, S], f32, tag="r")
                    nc.scalar.activation(r[:], p_r[:], AF.Sigmoid, bias=W.c_r[:])
                    z = sb.tile([128, S], f32, tag="z")
                    nc.scalar.activation(z[:], p_z[:], AF.Sigmoid, bias=W.c_z[:])
                    hn = sb.tile([128, S], f32, tag="hn")
                    nc.scalar.activation(hn[:], p_n[:], AF.Identity, bias=W.bhn[:])
                    rn = sb.tile([128, S], f32, tag="rn")
                    nc.vector.tensor_tensor(out=rn[:], in0=r[:], in1=hn[:],
                                            op=OP.mult)
                    n = sb.tile([128, S], f32, tag="n")
                    nc.scalar.activation(n[:], rn[:], AF.Tanh, bias=W.gin[:])
                    d = sb.tile([128, S], f32, tag="d")
                    nc.vector.tensor_tensor(out=d[:], in0=stT, in1=n[:],
                                            op=OP.subtract)
                    zd = sb.tile([128, S], f32, tag="zd")
                    nc.vector.tensor_tensor(out=zd[:], in0=z[:], in1=d[:],
                                            op=OP.mult)
                    ns = sb.tile([128, S], f32, tag="ns")
                    nc.vector.tensor_tensor(out=ns[:], in0=n[:], in1=zd[:],
                                            op=OP.add)
                    sstate = sb.tile([128, S], bf16, tag="sstate")
                    nc.vector.tensor_scalar_mul(sstate[:], ns[:],
                                                W.dect[:, 0:1])

                t1 = sb.tile([128, S], bf16, tag="t1")
                nc.vector.tensor_tensor(out=t1[:], in0=sstate[:], in1=memT,
                                        op=OP.add)
                p_h1 = ps2.tile([128, S], f32, tag="h1")
                nc.tensor.matmul(p_h1[:], lhsT=W.wfa[:], rhs=fA,
                                 start=True, stop=False)
                nc.tensor.matmul(p_h1[:], lhsT=W.wfb[:], rhs=fB,
                                 start=False, stop=False)
                nc.tensor.matmul(p_h1[:], lhsT=W.w1t[:], rhs=t1[:],
                                 start=False, stop=True)
                h1 = sb.tile([128, S], bf16, tag="h1s")
                nc.scalar.activation(h1[:], p_h1[:], AF.Relu, bias=W.b1p[:],
                                     scale=1.0 / 16.0)
                o_sl = p_o[0:64, :] if s == 0 else p_o[64:128, :]
                nc.tensor.matmul(o_sl, lhsT=W.w2t[:], rhs=h1[:],
                                 start=True, stop=True)

            def stream(gp, sb, ps, ps2, pso, table_ap, idx_tile, n_occ,
                       col0h, touched):
                """Process one (chunk, touched?) stream of n_occ occurrences
                in gathers of G; outputs to out[:, col0h : col0h+n_occ/2]."""
                n_g = n_occ // G
                for g in range(n_g):
                    X = gp.tile([128, 3, G], bf16, tag=f"gath{touched}")
                    nc.gpsimd.dma_gather(
                        out_ap=X[:],
                        in_ap=table_ap[:],
                        idxs_ap=idx_tile[:, bass.ds(g * G // 16, G // 16)],
                        num_idxs=G,
                        num_idxs_reg=G,
                        elem_size=ROW,
                        transpose=True,
                    )
                    p_o = pso.tile([128, S], f32, tag=f"po{touched}")
                    for s in range(2):
                        supertile(sb, ps, ps2, p_o, X, s, touched)
                    osb = gp.tile([OC, S], bf16, tag=f"osb{touched}")
                    nc.vector.tensor_scalar_add(osb[:], p_o[0:OC, :],
                                                W.b2v2[:, 0:1])
                    nc.sync.dma_start(
                        out[:, bass.ds(col0h + g * S, S)], osb[:])

            # One deep gather pool shared by both phases so the Pool engine
            # streams descriptor generation continuously across the
            # touched->untouched transition. Touched first: its serial GRU
            # chain drains while untouched gathers already issue.
            with tc.tile_pool(name="gp", bufs=8) as gp:
                with tc.tile_pool(name="sbt", bufs=2) as sb, \
                     tc.tile_pool(name="pst", bufs=1, space="PSUM") as ps, \
                     tc.tile_pool(name="pst2", bufs=1, space="PSUM") as ps2, \
                     tc.tile_pool(name="psot", bufs=1, space="PSUM") as pso:
                    stream(gp, sb, ps, ps2, pso, tab_a, W.tidx_a, t_pad,
                           u_pad // 2, True)
                    stream(gp, sb, ps, ps2, pso, tab_b, W.tidx_b, t_pad,
                           u_pad + t_pad // 2, True)
                with tc.tile_pool(name="sbu", bufs=4) as sb, \
                     tc.tile_pool(name="psu", bufs=2, space="PSUM") as ps, \
                     tc.tile_pool(name="psu2", bufs=3, space="PSUM") as ps2, \
                     tc.tile_pool(name="psou", bufs=3, space="PSUM") as pso:
                    stream(gp, sb, ps, ps2, pso, tab_a, W.uidx_a, u_pad,
                           0, False)
                    stream(gp, sb, ps, ps2, pso, tab_b, W.uidx_b, u_pad,
                           (u_pad + t_pad) // 2, False)

    nc.compile()
    return nc


@functools.lru_cache(maxsize=4)
def _cached_program(u_pad, t_pad, ch):
    return build_program(u_pad, t_pad, ch)


def _round_up(x, m):
    return ((x + m - 1) // m) * m


def _prepare(label_nodes, src, dst, t, msg, memory_buf, node_state, last_seen,
             node_features, decay, W_msg, b_msg, W_ih, W_hh, b_ih, b_hh,
             W_feat, b_feat, W1, b1, W2, b2, current_time):
    """Host-side routing/packing. Returns (in_maps, meta)."""
    label_nodes = np.asarray(label_nodes)

    # ---- host: event-level scalars (O(1) work) ----
    t0 = float(np.asarray(t)[0])
    T = float(current_time)
    rdecay = max(float(decay), 0.0)
    event_msg = msg[0].astype(np.float64) @ W_msg.T.astype(np.float64) + b_msg
    gi = event_msg @ W_ih.T.astype(np.float64) + b_ih  # [3H], includes b_ih
    gi = gi.astype(np.float32)
    dec_t = np.float32(np.exp(-rdecay * (T - t0)))

    # ---- host: routing (dedup to unique label nodes) ----
    touched_nodes = np.unique(np.concatenate([src, dst]))
    uniq_vals, inv = np.unique(label_nodes, return_inverse=True)
    is_t = np.isin(uniq_vals, touched_nodes)
    chunk_id = uniq_vals // CH            # 0..15
    local = (uniq_vals % CH).astype(np.int16)

    key = chunk_id.astype(np.int64) * 2 + is_t
    order = np.argsort(key, kind="stable")
    counts = np.bincount(key, minlength=NCHUNK * 2)
    u_counts = counts[0::2]
    t_counts = counts[1::2]
    u_pad = max(_round_up(int(u_counts.max()), G), G)
    t_pad = max(_round_up(int(t_counts.max()), G), G)

    starts = np.zeros(NCHUNK * 2 + 1, dtype=np.int64)
    np.cumsum(counts, out=starts[1:])

    # ---- host: packed 768B-row table ----
    tab8 = np.zeros((N, 2 * ROW), dtype=np.uint8)
    tab8[:, 0:256] = memory_buf.astype(BF16).view(np.uint8)
    tab8[:, 256:512] = node_state.astype(BF16).view(np.uint8)
    blk = np.zeros((N, 256), dtype=np.uint8)
    f8 = (node_features * FSC).astype(FP8).view(np.uint8)
    dec_pre = np.exp(rdecay * last_seen.astype(np.float64)).astype(BF16)
    blk[:, 0:2] = dec_pre.reshape(-1, 1).view(np.uint8)
    blk[:, 2:256:2] = f8[:, 1:128]        # featA: even bytes, units 1..127
    blk[:, 3:91:2] = f8[:, 128:172]       # featB: odd bytes, units 1..44
    blk[:, 91] = f8[:, 0]                 # feat[0]: odd byte, unit 45
    tab8[:, 512:768] = blk
    tab = tab8.view(BF16)                 # [N, 384] bf16-typed bytes

    # ---- host: weights / aux ----
    def bfc(x):
        return np.ascontiguousarray(x, dtype=BF16)

    def f32c(x):
        return np.ascontiguousarray(x, dtype=np.float32).reshape(-1, 1)

    WcT = (W1 @ W_feat).T * FSC  # [F, H]; W_feat folded through W1, x4
    z1 = np.zeros((1, H), np.float32)
    wfa = np.concatenate([z1, WcT[1:128]], axis=0)
    wfb = np.concatenate([z1, WcT[128:172], WcT[0:1]], axis=0)  # [46, H]
    b2v2 = np.zeros(OC, np.float32)
    b2v2[0:C] = b2
    b2v2[64:64 + C] = b2
    aux = {
        "wfa": np.ascontiguousarray(wfa, dtype=FP8),
        "wfb": np.ascontiguousarray(wfb, dtype=FP8),
        "w1t": bfc(W1.T * (FSC * FSC)),
        "w2t": bfc(np.concatenate(
            [W2.T, np.zeros((H, 64 - C), np.float32)], axis=1)),
        "whhrt": bfc(W_hh[0:128].T),
        "whhzt": bfc(W_hh[128:256].T),
        "whhnt": bfc(W_hh[256:384].T),
        "ones1": np.ones((1, 128), dtype=BF16),
        "b1p": f32c(b1 + W1 @ b_feat),
        "b2v2": f32c(b2v2),
        "c_r": f32c(gi[0:128] + b_hh[0:128]),
        "c_z": f32c(gi[128:256] + b_hh[128:256]),
        "gin": f32c(gi[256:384]),
        "bhn": f32c(b_hh[256:384]),
        "dect": np.full((128, 1), dec_t, dtype=np.float32),
        "dtc": np.full((128, 1), np.exp(-rdecay * T), dtype=np.float32),
    }

    # ---- host: per-core input maps ----
    in_maps = []
    group_uids = {}  # (chunk, touched) -> unique-label ids in device order
    for ci in range(NCHUNK):
        for tf in (0, 1):
            k = ci * 2 + tf
            group_uids[(ci, tf)] = order[starts[k]:starts[k + 1]]

    def idx_input(ci, tf, pad):
        uids = group_uids[(ci, tf)]
        li = np.zeros(pad, dtype=np.int16)
        li[:uids.shape[0]] = local[uids]
        return _wrap_idxs(li)

    for core in range(NCORES):
        ca, cb = 2 * core, 2 * core + 1
        im = dict(aux)
        im["tab_a"] = tab[ca * CH:(ca + 1) * CH]
        im["tab_b"] = tab[cb * CH:(cb + 1) * CH]
        im["uidx_a"] = idx_input(ca, 0, u_pad)
        im["uidx_b"] = idx_input(cb, 0, u_pad)
        im["tidx_a"] = idx_input(ca, 1, t_pad)
        im["tidx_b"] = idx_input(cb, 1, t_pad)
        in_maps.append(im)

    # column (within a core's virtual [C, totcol] output) of each unique
    # label node; _finish first unpacks the [114, totcol/2] device layout.
    totcol = 2 * (u_pad + t_pad)
    col_of_uniq = np.empty(uniq_vals.shape[0], dtype=np.int64)
    for ci in range(NCHUNK):
        core = ci // 2
        for tf in (0, 1):
            uids = group_uids[(ci, tf)]
            if (ci % 2) == 0:
                c0 = 0 if tf == 0 else u_pad
            else:
                c0 = (u_pad + t_pad) if tf == 0 else (2 * u_pad + t_pad)
            col_of_uniq[uids] = core * totcol + c0 + np.arange(uids.shape[0])

    meta = {"u_pad": u_pad, "t_pad": t_pad, "col_of_uniq": col_of_uniq,
            "inv": inv, "nb": label_nodes.shape[0]}
    return in_maps, meta


def _finish(core_outs, meta):
    """Unpack [114, tch] col-tiled outputs and map back to label order."""
    outs50 = []
    for o in core_outs:
        o = np.asarray(o, dtype=np.float32)
        nh = o.shape[1]
        arr = o.reshape(OC, nh // S, S)
        st = np.stack([arr[0:C], arr[64:64 + C]], axis=2)  # [C, nh/S, 2, S]
        outs50.append(st.reshape(C, 2 * nh))
    combined = np.concatenate(outs50, axis=1)  # [C, NCORES*totcol]
    return np.ascontiguousarray(
        combined[:, meta["col_of_uniq"][meta["inv"]]].T)


def kernel(**inputs):
    inputs = {k: np.asarray(v) for k, v in inputs.items()}
    in_maps, meta = _prepare(**inputs)
    nc = _cached_program(meta["u_pad"], meta["t_pad"], CH)
    res = run_bass_kernel_spmd(nc, in_maps, core_ids=list(range(NCORES)))
    return _finish([r["out"] for r in res.results], meta)


# revision 37
# speedup vs baseline: 1.1800x; 1.0104x over previous
"""DyRep classifier Bass kernel for 8 Trainium2 NeuronCores.

Strategy (self-contained; shapes hardcoded for the target problem):
  - The output depends only on per-label-node rows of (memory_buf,
    node_state[post-event], last_seen[post-event], node_features).
  - Host packs the four per-node tables into one 768-byte row per node
    (vs 1024B for the bf16-only layout):
      block0 (256B): memory_buf bf16 — DVE operand.
      block1 (256B): node_state bf16 — GRU matmul rhs / DVE operand.
      block2 (256B): dec_pre=exp(relu(decay)*last_seen) bf16 at u16-unit
        0, feat*4 as fp8 pairs on units 1..127 (even bytes feat[1:128],
        odd bytes feat[128:172] on units 1..44 and feat[0] on unit 45).
    dec_pre is in [1,1.65) so BOTH its bytes decode as finite fp8 — the
    feat matmuls span it with zero weight rows without NaN risk. The
    feat matmuls run fp8 x fp8 (weights also *4; HW runs mixed
    bf16 x fp8 matmuls incorrectly, and fp8 weights at natural scale
    would be subnormal). The combined x16 is undone for free by the
    ACT scale on the h1 relu; W1 is pre-multiplied by 16 to match.
    No exp on device: dec = dec_pre * exp(-rd*T) via an ACT scale on a
    [1,S] slice, broadcast across partitions by a K=1 ones-matmul.
    Sharded row-wise into 16 chunks of 31250 rows (int16-indexable);
    each of the 8 cores owns 2 chunks.
  - Host routes each unique label node to its owner (core, chunk) and
    splits into "untouched" / "touched" (touched = node hit by the
    event batch, needs the GRU update applied on the fly).
  - Device per core: dma_gather(transpose=True) delivers rows
    feature-major. h1 psum accumulates 3 matmuls (featA fp8, featB fp8,
    W1@(mem + state*dec)), ACT relu with fused bias+scale, then W2
    (zero-padded to 64 rows) col-tiled so both supertiles of a gather
    land in ONE [128, 448] psum bank (s0 -> partitions 0:64,
    s1 -> 64:128), one DVE bias-copy, bf16 half-width output.
  - Host unpermutes the per-core outputs back to label order.
"""

import functools
import os
import numpy as np
import ml_dtypes

import concourse.bass as bass
import concourse.mybir as mybir
import concourse.tile as tile
from concourse import bacc
from concourse.bass_utils import run_bass_kernel_spmd

BF16 = ml_dtypes.bfloat16
FP8 = ml_dtypes.float8_e4m3

# Problem dims (fixed by the task)
N = 500000
H = 128
F = 172
C = 50
B = 200000

NCORES = 8
NCHUNK = 16                  # index chunks (int16 addressing limit)
CH = N // NCHUNK             # 31250 rows per chunk
ROW = 384                    # bf16-unit elements per packed row (768 bytes)
G = int(os.environ.get("K_G", "896"))   # occurrences per gather
# (transpose dma_gather num_idxs is HW-capped at 1016: 896 largest %128)
S = G // 2                   # supertile: occurrences per compute slice
FSC = 4.0                    # fp8 scale for feat AND its weights (x16 total)
OC = 114                     # packed output partitions (2 x C, col-tiled)

f32 = mybir.dt.float32
bf16 = mybir.dt.bfloat16
fp8 = mybir.dt.float8e4
i16 = mybir.dt.int16
AF = mybir.ActivationFunctionType
OP = mybir.AluOpType


def _wrap_idxs(idx: np.ndarray) -> np.ndarray:
    """Wrap a flat int16 index list into the [128, n/16] SWDGE layout:
    element j at [j%16, j//16], replicated into all 8 16-partition groups."""
    n = idx.shape[0]
    assert n % 16 == 0
    cols = n // 16
    t = np.empty((128, cols), dtype=np.int16)
    blk = idx.reshape(cols, 16).T  # [16, cols]
    for k in range(8):
        t[k * 16:(k + 1) * 16, :] = blk
    return t


def build_program(u_pad: int, t_pad: int, ch: int = CH):
    """Build + compile the SPMD Bass program. Cached by padded sizes."""
    nc = bacc.Bacc("TRN2", target_bir_lowering=False, debug=False,
                   num_devices=NCORES)

    dt_in = {}

    def din(name, shape, dt):
        dt_in[name] = nc.dram_tensor(name, shape, dt, kind="ExternalInput").ap()
        return dt_in[name]

    tab_a = din("tab_a", (ch, ROW), bf16)
    tab_b = din("tab_b", (ch, ROW), bf16)
    uidx_a = din("uidx_a", (128, u_pad // 16), i16)
    uidx_b = din("uidx_b", (128, u_pad // 16), i16)
    tidx_a = din("tidx_a", (128, t_pad // 16), i16)
    tidx_b = din("tidx_b", (128, t_pad // 16), i16)

    wfab = din("wfab", (128, 2, 128), fp8)   # DoubleRow planes: [k,0,:] even
    #   byte weights (feat[k], row0 zero), [k,1,:] odd byte weights
    #   (feat[127+k] for k=1..44, feat[0] at k=45, zero elsewhere); all *4
    w1t = din("w1t", (128, 128), bf16)       # W1.T * 16
    w2t = din("w2t", (128, 64), bf16)        # [W2.T | zeros] (64 cols)
    whhrt = din("whhrt", (128, 128), bf16)   # W_hh[0:128].T
    whhzt = din("whhzt", (128, 128), bf16)   # W_hh[128:256].T
    whhnt = din("whhnt", (128, 128), bf16)   # W_hh[256:384].T
    ones1 = din("ones1", (1, 128), bf16)
    b1p = din("b1p", (128, 1), f32)          # b1 + W1 @ b_feat
    b2v2 = din("b2v2", (OC, 1), f32)         # [b2; 0*14; b2]
    c_r = din("c_r", (128, 1), f32)          # gi_r + b_hh_r
    c_z = din("c_z", (128, 1), f32)          # gi_z + b_hh_z
    gin = din("gin", (128, 1), f32)          # gi_n
    bhn = din("bhn", (128, 1), f32)          # b_hh_n
    dect = din("dect", (128, 1), f32)        # exp(-relu(decay)*(T-t0))
    dtc = din("dtc", (128, 1), f32)          # exp(-relu(decay)*T)

    tch = (u_pad + t_pad)                    # out columns per core (half)
    out = nc.dram_tensor("out", (OC, tch), bf16, kind="ExternalOutput").ap()

    class W:
        pass

    with tile.TileContext(nc) as tc:
        with tc.tile_pool(name="wpool", bufs=1) as wp:
            for name in ("tidx_a", "tidx_b", "uidx_a", "uidx_b",
                         "wfab", "w1t", "w2t", "whhrt", "whhzt",
                         "whhnt", "ones1", "b1p", "b2v2", "c_r", "c_z", "gin",
                         "bhn", "dect", "dtc"):
                ap = dt_in[name]
                t = wp.tile(list(ap.shape), ap.dtype, tag=name)
                nc.sync.dma_start(t[:], ap[:])
                setattr(W, name, t)

            def fab_mm(p_h1, X8, s):
                """Open the h1 psum group: featA+featB in ONE DoubleRow
                matmul consuming the raw fp8 byte pairs of block2."""
                rhs = X8[:, 2, bass.ds(2 * s * S, 2 * S)].rearrange(
                    "p (n two) -> p two n", two=2)
                nc.tensor.matmul(p_h1[:], lhsT=W.wfab[:], rhs=rhs,
                                 start=True, stop=False,
                                 perf_mode=mybir.MatmulPerfMode.DoubleRow)

            def finish_tile(sb, ps2, p_o, X, s, p_h1, sstate):
                """Close the h1 group (W1 matmul), relu, W2 into the
                col-tiled p_o half."""
                sl = bass.ds(s * S, S)
                memT = X[:, 0, sl]
                t1 = sb.tile([128, S], bf16, tag="t1")
                nc.vector.tensor_tensor(out=t1[:], in0=sstate[:], in1=memT,
                                        op=OP.add)
                nc.tensor.matmul(p_h1[:], lhsT=W.w1t[:], rhs=t1[:],
                                 start=False, stop=True)
                h1 = sb.tile([128, S], bf16, tag="h1s")
                nc.scalar.activation(h1[:], p_h1[:], AF.Relu, bias=W.b1p[:],
                                     scale=1.0 / 16.0)
                o_sl = p_o[0:64, :] if s == 0 else p_o[64:128, :]
                nc.tensor.matmul(o_sl, lhsT=W.w2t[:], rhs=h1[:],
                                 start=True, stop=True)

            def untouched_gather(sb, ps, ps2, p_o, X):
                """Both supertiles of one gather, phase-split so the
                gather-ready matmuls (ones-bcast, feat DoubleRow) issue
                before the DVE-dependent W1/W2 chain."""
                X8 = X[:].bitcast(fp8)
                p_dec, p_h1 = [], []
                for s in range(2):
                    dpT = X[0:1, 2, bass.ds(s * S, S)]
                    dec_b = sb.tile([1, S], bf16, tag="dec_b")
                    nc.scalar.activation(dec_b[:], dpT, AF.Identity,
                                         scale=W.dtc[0:1, :])
                    pd = ps.tile([128, S], f32, tag="dec")
                    nc.tensor.matmul(pd[:], lhsT=W.ones1[:],
                                     rhs=dec_b[:], start=True, stop=True)
                    p_dec.append(pd)
                    ph = ps2.tile([128, S], f32, tag="h1")
                    fab_mm(ph, X8, s)
                    p_h1.append(ph)
                for s in range(2):
                    stT = X[:, 1, bass.ds(s * S, S)]
                    sstate = sb.tile([128, S], bf16, tag="sstate")
                    nc.vector.tensor_tensor(out=sstate[:], in0=stT,
                                            in1=p_dec[s][:], op=OP.mult)
                    finish_tile(sb, ps2, p_o, X, s, p_h1[s], sstate)

            def touched_supertile(sb, ps, ps2, p_o, X, s):
                sl = bass.ds(s * S, S)
                X8 = X[:].bitcast(fp8)
                stT = X[:, 1, sl]
                if True:
                    p_r = ps.tile([128, S], f32, tag="gr")
                    nc.tensor.matmul(p_r[:], lhsT=W.whhrt[:], rhs=stT,
                                     start=True, stop=True)
                    p_z = ps.tile([128, S], f32, tag="gz")
                    nc.tensor.matmul(p_z[:], lhsT=W.whhzt[:], rhs=stT,
                                     start=True, stop=True)
                    p_n = ps.tile([128, S], f32, tag="gn")
                    nc.tensor.matmul(p_n[:], lhsT=W.whhnt[:], rhs=stT,
                                     start=True, stop=True)
                    r = sb.tile([128, S], f32, tag="r")
                    nc.scalar.activation(r[:], p_r[:], AF.Sigmoid, bias=W.c_r[:])
                    z = sb.tile([128, S], f32, tag="z")
                    nc.scalar.activation(z[:], p_z[:], AF.Sigmoid, bias=W.c_z[:])
                    hn = sb.tile([128, S], f32, tag="hn")
                    nc.scalar.activation(hn[:], p_n[:], AF.Identity, bias=W.bhn[:])
                    rn = sb.tile([128, S], f32, tag="rn")
                    nc.vector.tensor_tensor(out=rn[:], in0=r[:], in1=hn[:],
                                            op=OP.mult)
                    n = sb.tile([128, S], f32, tag="n")
                    nc.scalar.activation(n[:], rn[:], AF.Tanh, bias=W.gin[:])
                    d = sb.tile([128, S], f32, tag="d")
                    nc.vector.tensor_tensor(out=d[:], in0=stT, in1=n[:],
                                            op=OP.subtract)
                    zd = sb.tile([128, S], f32, tag="zd")
                    nc.vector.tensor_tensor(out=zd[:], in0=z[:], in1=d[:],
                                            op=OP.mult)
                    ns = sb.tile([128, S], f32, tag="ns")
                    nc.vector.tensor_tensor(out=ns[:], in0=n[:], in1=zd[:],
                                            op=OP.add)
                    sstate = sb.tile([128, S], bf16, tag="sstate")
                    nc.vector.tensor_scalar_mul(sstate[:], ns[:],
                                                W.dect[:, 0:1])

                p_h1 = ps2.tile([128, S], f32, tag="h1")
                fab_mm(p_h1, X8, s)
                finish_tile(sb, ps2, p_o, X, s, p_h1, sstate)

            def stream(gp, sb, ps, ps2, pso, table_ap, idx_tile, n_occ,
                       col0h, touched):
                """Process one (chunk, touched?) stream of n_occ occurrences
                in gathers of G; outputs to out[:, col0h : col0h+n_occ/2]."""
                n_g = n_occ // G
                for g in range(n_g):
                    X = gp.tile([128, 3, G], bf16, tag=f"gath{touched}")
                    nc.gpsimd.dma_gather(
                        out_ap=X[:],
                        in_ap=table_ap[:],
                        idxs_ap=idx_tile[:, bass.ds(g * G // 16, G // 16)],
                        num_idxs=G,
                        num_idxs_reg=G,
                        elem_size=ROW,
                        transpose=True,
                    )
                    p_o = pso.tile([128, S], f32, tag=f"po{touched}")
                    if touched:
                        for s in range(2):
                            touched_supertile(sb, ps, ps2, p_o, X, s)
                    else:
                        untouched_gather(sb, ps, ps2, p_o, X)
                    osb = gp.tile([OC, S], bf16, tag=f"osb{touched}")
                    nc.vector.tensor_scalar_add(osb[:], p_o[0:OC, :],
                                                W.b2v2[:, 0:1])
                    nc.sync.dma_start(
                        out[:, bass.ds(col0h + g * S, S)], osb[:])

            # One deep gather pool shared by both phases so the Pool engine
            # streams descriptor generation continuously across the
            # touched->untouched transition. Touched first: its serial GRU
            # chain drains while untouched gathers already issue.
            with tc.tile_pool(name="gp", bufs=8) as gp:
                with tc.tile_pool(name="sbt", bufs=2) as sb, \
                     tc.tile_pool(name="pst", bufs=1, space="PSUM") as ps, \
                     tc.tile_pool(name="pst2", bufs=1, space="PSUM") as ps2, \
                     tc.tile_pool(name="psot", bufs=1, space="PSUM") as pso:
                    stream(gp, sb, ps, ps2, pso, tab_a, W.tidx_a, t_pad,
                           u_pad // 2, True)
                    stream(gp, sb, ps, ps2, pso, tab_b, W.tidx_b, t_pad,
                           u_pad + t_pad // 2, True)
                with tc.tile_pool(name="sbu", bufs=4) as sb, \
                     tc.tile_pool(name="psu", bufs=2, space="PSUM") as ps, \
                     tc.tile_pool(name="psu2", bufs=3, space="PSUM") as ps2, \
                     tc.tile_pool(name="psou", bufs=3, space="PSUM") as pso:
                    stream(gp, sb, ps, ps2, pso, tab_a, W.uidx_a, u_pad,
                           0, False)
                    stream(gp, sb, ps, ps2, pso, tab_b, W.uidx_b, u_pad,
                           (u_pad + t_pad) // 2, False)

    nc.compile()
    return nc


@functools.lru_cache(maxsize=4)
def _cached_program(u_pad, t_pad, ch):
    return build_program(u_pad, t_pad, ch)


def _round_up(x, m):
    return ((x + m - 1) // m) * m


def _prepare(label_nodes, src, dst, t, msg, memory_buf, node_state, last_seen,
             node_features, decay, W_msg, b_msg, W_ih, W_hh, b_ih, b_hh,
             W_feat, b_feat, W1, b1, W2, b2, current_time):
    """Host-side routing/packing. Returns (in_maps, meta)."""
    label_nodes = np.asarray(label_nodes)

    # ---- host: event-level scalars (O(1) work) ----
    t0 = float(np.asarray(t)[0])
    T = float(current_time)
    rdecay = max(float(decay), 0.0)
    event_msg = msg[0].astype(np.float64) @ W_msg.T.astype(np.float64) + b_msg
    gi = event_msg @ W_ih.T.astype(np.float64) + b_ih  # [3H], includes b_ih
    gi = gi.astype(np.float32)
    dec_t = np.float32(np.exp(-rdecay * (T - t0)))

    # ---- host: routing (dedup to unique label nodes) ----
    touched_nodes = np.unique(np.concatenate([src, dst]))
    uniq_vals, inv = np.unique(label_nodes, return_inverse=True)
    is_t = np.isin(uniq_vals, touched_nodes)
    chunk_id = uniq_vals // CH            # 0..15
    local = (uniq_vals % CH).astype(np.int16)

    key = chunk_id.astype(np.int64) * 2 + is_t
    order = np.argsort(key, kind="stable")
    counts = np.bincount(key, minlength=NCHUNK * 2)
    u_counts = counts[0::2]
    t_counts = counts[1::2]
    u_pad = max(_round_up(int(u_counts.max()), G), G)
    t_pad = max(_round_up(int(t_counts.max()), G), G)

    starts = np.zeros(NCHUNK * 2 + 1, dtype=np.int64)
    np.cumsum(counts, out=starts[1:])

    # ---- host: packed 768B-row table ----
    tab8 = np.zeros((N, 2 * ROW), dtype=np.uint8)
    tab8[:, 0:256] = memory_buf.astype(BF16).view(np.uint8)
    tab8[:, 256:512] = node_state.astype(BF16).view(np.uint8)
    blk = np.zeros((N, 256), dtype=np.uint8)
    f8 = (node_features * FSC).astype(FP8).view(np.uint8)
    dec_pre = np.exp(rdecay * last_seen.astype(np.float64)).astype(BF16)
    blk[:, 0:2] = dec_pre.reshape(-1, 1).view(np.uint8)
    blk[:, 2:256:2] = f8[:, 1:128]        # featA: even bytes, units 1..127
    blk[:, 3:91:2] = f8[:, 128:172]       # featB: odd bytes, units 1..44
    blk[:, 91] = f8[:, 0]                 # feat[0]: odd byte, unit 45
    tab8[:, 512:768] = blk
    tab = tab8.view(BF16)                 # [N, 384] bf16-typed bytes

    # ---- host: weights / aux ----
    def bfc(x):
        return np.ascontiguousarray(x, dtype=BF16)

    def f32c(x):
        return np.ascontiguousarray(x, dtype=np.float32).reshape(-1, 1)

    WcT = (W1 @ W_feat).T * FSC  # [F, H]; W_feat folded through W1, x4
    wfab = np.zeros((128, 2, H), np.float32)
    wfab[1:128, 0] = WcT[1:128]           # even-byte plane: feat[1:128]
    wfab[1:45, 1] = WcT[128:172]          # odd-byte plane: feat[128:172]
    wfab[45, 1] = WcT[0]                  # feat[0] at odd byte of unit 45
    b2v2 = np.zeros(OC, np.float32)
    b2v2[0:C] = b2
    b2v2[64:64 + C] = b2
    aux = {
        "wfab": np.ascontiguousarray(wfab, dtype=FP8),
        "w1t": bfc(W1.T * (FSC * FSC)),
        "w2t": bfc(np.concatenate(
            [W2.T, np.zeros((H, 64 - C), np.float32)], axis=1)),
        "whhrt": bfc(W_hh[0:128].T),
        "whhzt": bfc(W_hh[128:256].T),
        "whhnt": bfc(W_hh[256:384].T),
        "ones1": np.ones((1, 128), dtype=BF16),
        "b1p": f32c(b1 + W1 @ b_feat),
        "b2v2": f32c(b2v2),
        "c_r": f32c(gi[0:128] + b_hh[0:128]),
        "c_z": f32c(gi[128:256] + b_hh[128:256]),
        "gin": f32c(gi[256:384]),
        "bhn": f32c(b_hh[256:384]),
        "dect": np.full((128, 1), dec_t, dtype=np.float32),
        "dtc": np.full((128, 1), np.exp(-rdecay * T), dtype=np.float32),
    }

    # ---- host: per-core input maps ----
    in_maps = []
    group_uids = {}  # (chunk, touched) -> unique-label ids in device order
    for ci in range(NCHUNK):
        for tf in (0, 1):
            k = ci * 2 + tf
            group_uids[(ci, tf)] = order[starts[k]:starts[k + 1]]

    def idx_input(ci, tf, pad):
        uids = group_uids[(ci, tf)]
        li = np.zeros(pad, dtype=np.int16)
        li[:uids.shape[0]] = local[uids]
        return _wrap_idxs(li)

    for core in range(NCORES):
        ca, cb = 2 * core, 2 * core + 1
        im = dict(aux)
        im["tab_a"] = tab[ca * CH:(ca + 1) * CH]
        im["tab_b"] = tab[cb * CH:(cb + 1) * CH]
        im["uidx_a"] = idx_input(ca, 0, u_pad)
        im["uidx_b"] = idx_input(cb, 0, u_pad)
        im["tidx_a"] = idx_input(ca, 1, t_pad)
        im["tidx_b"] = idx_input(cb, 1, t_pad)
        in_maps.append(im)

    # column (within a core's virtual [C, totcol] output) of each unique
    # label node; _finish first unpacks the [114, totcol/2] device layout.
    totcol = 2 * (u_pad + t_pad)
    col_of_uniq = np.empty(uniq_vals.shape[0], dtype=np.int64)
    for ci in range(NCHUNK):
        core = ci // 2
        for tf in (0, 1):
            uids = group_uids[(ci, tf)]
            if (ci % 2) == 0:
                c0 = 0 if tf == 0 else u_pad
            else:
                c0 = (u_pad + t_pad) if tf == 0 else (2 * u_pad + t_pad)
            col_of_uniq[uids] = core * totcol + c0 + np.arange(uids.shape[0])

    meta = {"u_pad": u_pad, "t_pad": t_pad, "col_of_uniq": col_of_uniq,
            "inv": inv, "nb": label_nodes.shape[0]}
    return in_maps, meta


def _finish(core_outs, meta):
    """Unpack [114, tch] col-tiled outputs and map back to label order."""
    outs50 = []
    for o in core_outs:
        o = np.asarray(o, dtype=np.float32)
        nh = o.shape[1]
        arr = o.reshape(OC, nh // S, S)
        st = np.stack([arr[0:C], arr[64:64 + C]], axis=2)  # [C, nh/S, 2, S]
        outs50.append(st.reshape(C, 2 * nh))
    combined = np.concatenate(outs50, axis=1)  # [C, NCORES*totcol]
    return np.ascontiguousarray(
        combined[:, meta["col_of_uniq"][meta["inv"]]].T)


def kernel(**inputs):
    inputs = {k: np.asarray(v) for k, v in inputs.items()}
    in_maps, meta = _prepare(**inputs)
    nc = _cached_program(meta["u_pad"], meta["t_pad"], CH)
    res = run_bass_kernel_spmd(nc, in_maps, core_ids=list(range(NCORES)))
    return _finish([r["out"] for r in res.results], meta)


# revision 39
# speedup vs baseline: 1.7980x; 1.5237x over previous
"""DyRep classifier Bass kernel for 8 Trainium2 NeuronCores.

Strategy (self-contained; shapes hardcoded for the target problem):
  - The output depends only on per-label-node rows of (memory_buf,
    node_state[post-event], last_seen[post-event], node_features).
  - Host packs the four per-node tables into one 768-byte row per node
    (vs 1024B for the bf16-only layout):
      block0 (256B): memory_buf bf16 — DVE operand.
      block1 (256B): node_state bf16 — GRU matmul rhs / DVE operand.
      block2 (256B): dec_pre=exp(relu(decay)*last_seen) bf16 at u16-unit
        0, feat*4 as fp8 pairs on units 1..127 (even bytes feat[1:128],
        odd bytes feat[128:172] on units 1..44 and feat[0] on unit 45).
    dec_pre is in [1,1.65) so BOTH its bytes decode as finite fp8 — the
    feat matmuls span it with zero weight rows without NaN risk. The
    feat matmuls run fp8 x fp8 (weights also *4; HW runs mixed
    bf16 x fp8 matmuls incorrectly, and fp8 weights at natural scale
    would be subnormal). The combined x16 is undone for free by the
    ACT scale on the h1 relu; W1 is pre-multiplied by 16 to match.
    No exp on device: dec = dec_pre * exp(-rd*T) via an ACT scale on a
    [1,S] slice, broadcast across partitions by a K=1 ones-matmul.
    Sharded row-wise into 16 chunks of 31250 rows (int16-indexable);
    each of the 8 cores owns 2 chunks.
  - Host routes each unique label node to its owner (core, chunk) and
    splits into "untouched" / "touched" (touched = node hit by the
    event batch, needs the GRU update applied on the fly).
  - Device per core: dma_gather(transpose=True) delivers rows
    feature-major. h1 psum accumulates 3 matmuls (featA fp8, featB fp8,
    W1@(mem + state*dec)), ACT relu with fused bias+scale, then W2
    (zero-padded to 64 rows) col-tiled so both supertiles of a gather
    land in ONE [128, 448] psum bank (s0 -> partitions 0:64,
    s1 -> 64:128), one DVE bias-copy, bf16 half-width output.
  - Host unpermutes the per-core outputs back to label order.
"""

import functools
import os
import numpy as np
import ml_dtypes

import concourse.bass as bass
import concourse.mybir as mybir
import concourse.tile as tile
from concourse import bacc
from concourse.bass_utils import run_bass_kernel_spmd

BF16 = ml_dtypes.bfloat16
FP8 = ml_dtypes.float8_e4m3

# Problem dims (fixed by the task)
N = 500000
H = 128
F = 172
C = 50
B = 200000

NCORES = 8
NCHUNK = 16                  # index chunks (int16 addressing limit)
CH = N // NCHUNK             # 31250 rows per chunk
ROW = 384                    # bf16-unit elements per packed row (768 bytes)
G = int(os.environ.get("K_G", "896"))   # occurrences per gather
# (transpose dma_gather num_idxs is HW-capped at 1016: 896 largest %128)
S = G // 2                   # supertile: occurrences per compute slice
FSC = 4.0                    # fp8 scale for feat AND its weights (x16 total)
OC = 114                     # packed output partitions (2 x C, col-tiled)
NSWQ = int(os.environ.get("K_NSWQ", "2"))  # SWDGE queues: alternate gathers
# across queues so one gather's ring drain overlaps the next gather's
# descriptor generation (each queue has its own 256-desc ring).

f32 = mybir.dt.float32
bf16 = mybir.dt.bfloat16
fp8 = mybir.dt.float8e4
i16 = mybir.dt.int16
AF = mybir.ActivationFunctionType
OP = mybir.AluOpType


def _wrap_idxs(idx: np.ndarray) -> np.ndarray:
    """Wrap a flat int16 index list into the [128, n/16] SWDGE layout:
    element j at [j%16, j//16], replicated into all 8 16-partition groups."""
    n = idx.shape[0]
    assert n % 16 == 0
    cols = n // 16
    t = np.empty((128, cols), dtype=np.int16)
    blk = idx.reshape(cols, 16).T  # [16, cols]
    for k in range(8):
        t[k * 16:(k + 1) * 16, :] = blk
    return t


def build_program(u_pad: int, t_pad: int, ch: int = CH):
    """Build + compile the SPMD Bass program. Cached by padded sizes."""
    nc = bacc.Bacc("TRN2", target_bir_lowering=False, debug=False,
                   num_devices=NCORES, num_swdge_queues=NSWQ)

    dt_in = {}

    def din(name, shape, dt):
        dt_in[name] = nc.dram_tensor(name, shape, dt, kind="ExternalInput").ap()
        return dt_in[name]

    tab_a = din("tab_a", (ch, ROW), bf16)
    tab_b = din("tab_b", (ch, ROW), bf16)
    uidx_a = din("uidx_a", (128, u_pad // 16), i16)
    uidx_b = din("uidx_b", (128, u_pad // 16), i16)
    tidx_a = din("tidx_a", (128, t_pad // 16), i16)
    tidx_b = din("tidx_b", (128, t_pad // 16), i16)

    wfab = din("wfab", (128, 2, 128), fp8)   # DoubleRow planes: [k,0,:] even
    #   byte weights (feat[k], row0 zero), [k,1,:] odd byte weights
    #   (feat[127+k] for k=1..44, feat[0] at k=45, zero elsewhere); all *4
    w1t = din("w1t", (128, 128), bf16)       # W1.T * 16
    w2t = din("w2t", (128, 64), bf16)        # [W2.T | zeros] (64 cols)
    whhrt = din("whhrt", (128, 128), bf16)   # W_hh[0:128].T
    whhzt = din("whhzt", (128, 128), bf16)   # W_hh[128:256].T
    whhnt = din("whhnt", (128, 128), bf16)   # W_hh[256:384].T
    ones1 = din("ones1", (1, 128), bf16)
    b1p = din("b1p", (128, 1), f32)          # b1 + W1 @ b_feat
    b2v2 = din("b2v2", (OC, 1), f32)         # [b2; 0*14; b2]
    c_r = din("c_r", (128, 1), f32)          # gi_r + b_hh_r
    c_z = din("c_z", (128, 1), f32)          # gi_z + b_hh_z
    gin = din("gin", (128, 1), f32)          # gi_n
    bhn = din("bhn", (128, 1), f32)          # b_hh_n
    dect = din("dect", (128, 1), f32)        # exp(-relu(decay)*(T-t0))
    dtc = din("dtc", (128, 1), f32)          # exp(-relu(decay)*T)

    tch = (u_pad + t_pad)                    # out columns per core (half)
    out = nc.dram_tensor("out", (OC, tch), bf16, kind="ExternalOutput").ap()

    class W:
        pass

    with tile.TileContext(nc) as tc:
        with tc.tile_pool(name="wpool", bufs=1) as wp:
            for name in ("tidx_a", "tidx_b", "uidx_a", "uidx_b",
                         "wfab", "w1t", "w2t", "whhrt", "whhzt",
                         "whhnt", "ones1", "b1p", "b2v2", "c_r", "c_z", "gin",
                         "bhn", "dect", "dtc"):
                ap = dt_in[name]
                t = wp.tile(list(ap.shape), ap.dtype, tag=name)
                nc.sync.dma_start(t[:], ap[:])
                setattr(W, name, t)

            def fab_mm(p_h1, X8, s):
                """Open the h1 psum group: featA+featB in ONE DoubleRow
                matmul consuming the raw fp8 byte pairs of block2."""
                rhs = X8[:, 2, bass.ds(2 * s * S, 2 * S)].rearrange(
                    "p (n two) -> p two n", two=2)
                nc.tensor.matmul(p_h1[:], lhsT=W.wfab[:], rhs=rhs,
                                 start=True, stop=False,
                                 perf_mode=mybir.MatmulPerfMode.DoubleRow)

            def finish_tile(sb, ps2, p_o, X, s, p_h1, sstate):
                """Close the h1 group (W1 matmul), relu, W2 into the
                col-tiled p_o half."""
                sl = bass.ds(s * S, S)
                memT = X[:, 0, sl]
                t1 = sb.tile([128, S], bf16, tag="t1")
                nc.vector.tensor_tensor(out=t1[:], in0=sstate[:], in1=memT,
                                        op=OP.add)
                nc.tensor.matmul(p_h1[:], lhsT=W.w1t[:], rhs=t1[:],
                                 start=False, stop=True)
                h1 = sb.tile([128, S], bf16, tag="h1s")
                nc.scalar.activation(h1[:], p_h1[:], AF.Relu, bias=W.b1p[:],
                                     scale=1.0 / 16.0)
                o_sl = p_o[0:64, :] if s == 0 else p_o[64:128, :]
                nc.tensor.matmul(o_sl, lhsT=W.w2t[:], rhs=h1[:],
                                 start=True, stop=True)

            def untouched_gather(sb, ps, ps2, p_o, X):
                """Both supertiles of one gather, phase-split so the
                gather-ready matmuls (ones-bcast, feat DoubleRow) issue
                before the DVE-dependent W1/W2 chain."""
                X8 = X[:].bitcast(fp8)
                p_dec, p_h1 = [], []
                for s in range(2):
                    dpT = X[0:1, 2, bass.ds(s * S, S)]
                    dec_b = sb.tile([1, S], bf16, tag="dec_b")
                    nc.scalar.activation(dec_b[:], dpT, AF.Identity,
                                         scale=W.dtc[0:1, :])
                    pd = ps.tile([128, S], f32, tag="dec")
                    nc.tensor.matmul(pd[:], lhsT=W.ones1[:],
                                     rhs=dec_b[:], start=True, stop=True)
                    p_dec.append(pd)
                    ph = ps2.tile([128, S], f32, tag="h1")
                    fab_mm(ph, X8, s)
                    p_h1.append(ph)
                for s in range(2):
                    stT = X[:, 1, bass.ds(s * S, S)]
                    sstate = sb.tile([128, S], bf16, tag="sstate")
                    nc.vector.tensor_tensor(out=sstate[:], in0=stT,
                                            in1=p_dec[s][:], op=OP.mult)
                    finish_tile(sb, ps2, p_o, X, s, p_h1[s], sstate)

            def touched_supertile(sb, ps, ps2, p_o, X, s):
                sl = bass.ds(s * S, S)
                X8 = X[:].bitcast(fp8)
                stT = X[:, 1, sl]
                if True:
                    p_r = ps.tile([128, S], f32, tag="gr")
                    nc.tensor.matmul(p_r[:], lhsT=W.whhrt[:], rhs=stT,
                                     start=True, stop=True)
                    p_z = ps.tile([128, S], f32, tag="gz")
                    nc.tensor.matmul(p_z[:], lhsT=W.whhzt[:], rhs=stT,
                                     start=True, stop=True)
                    p_n = ps.tile([128, S], f32, tag="gn")
                    nc.tensor.matmul(p_n[:], lhsT=W.whhnt[:], rhs=stT,
                                     start=True, stop=True)
                    r = sb.tile([128, S], f32, tag="r")
                    nc.scalar.activation(r[:], p_r[:], AF.Sigmoid, bias=W.c_r[:])
                    z = sb.tile([128, S], f32, tag="z")
                    nc.scalar.activation(z[:], p_z[:], AF.Sigmoid, bias=W.c_z[:])
                    hn = sb.tile([128, S], f32, tag="hn")
                    nc.scalar.activation(hn[:], p_n[:], AF.Identity, bias=W.bhn[:])
                    rn = sb.tile([128, S], f32, tag="rn")
                    nc.vector.tensor_tensor(out=rn[:], in0=r[:], in1=hn[:],
                                            op=OP.mult)
                    n = sb.tile([128, S], f32, tag="n")
                    nc.scalar.activation(n[:], rn[:], AF.Tanh, bias=W.gin[:])
                    d = sb.tile([128, S], f32, tag="d")
                    nc.vector.tensor_tensor(out=d[:], in0=stT, in1=n[:],
                                            op=OP.subtract)
                    zd = sb.tile([128, S], f32, tag="zd")
                    nc.vector.tensor_tensor(out=zd[:], in0=z[:], in1=d[:],
                                            op=OP.mult)
                    ns = sb.tile([128, S], f32, tag="ns")
                    nc.vector.tensor_tensor(out=ns[:], in0=n[:], in1=zd[:],
                                            op=OP.add)
                    sstate = sb.tile([128, S], bf16, tag="sstate")
                    nc.vector.tensor_scalar_mul(sstate[:], ns[:],
                                                W.dect[:, 0:1])

                p_h1 = ps2.tile([128, S], f32, tag="h1")
                fab_mm(p_h1, X8, s)
                finish_tile(sb, ps2, p_o, X, s, p_h1, sstate)

            gctr = [0]

            def stream(gp, sb, ps, ps2, pso, table_ap, idx_tile, n_occ,
                       col0h, touched):
                """Process one (chunk, touched?) stream of n_occ occurrences
                in gathers of G; outputs to out[:, col0h : col0h+n_occ/2]."""
                n_g = n_occ // G
                for g in range(n_g):
                    X = gp.tile([128, 3, G], bf16, tag=f"gath{touched}")
                    nc.gpsimd.dma_gather(
                        out_ap=X[:],
                        in_ap=table_ap[:],
                        idxs_ap=idx_tile[:, bass.ds(g * G // 16, G // 16)],
                        num_idxs=G,
                        num_idxs_reg=G,
                        elem_size=ROW,
                        transpose=True,
                        queue_num=gctr[0] % NSWQ,
                    )
                    gctr[0] += 1
                    p_o = pso.tile([128, S], f32, tag=f"po{touched}")
                    if touched:
                        for s in range(2):
                            touched_supertile(sb, ps, ps2, p_o, X, s)
                    else:
                        untouched_gather(sb, ps, ps2, p_o, X)
                    osb = gp.tile([OC, S], bf16, tag=f"osb{touched}")
                    nc.vector.tensor_scalar_add(osb[:], p_o[0:OC, :],
                                                W.b2v2[:, 0:1])
                    nc.sync.dma_start(
                        out[:, bass.ds(col0h + g * S, S)], osb[:])

            # One deep gather pool shared by both phases so the Pool engine
            # streams descriptor generation continuously across the
            # touched->untouched transition. Touched first: its serial GRU
            # chain drains while untouched gathers already issue.
            with tc.tile_pool(name="gp", bufs=8) as gp:
                with tc.tile_pool(name="sbt", bufs=2) as sb, \
                     tc.tile_pool(name="pst", bufs=1, space="PSUM") as ps, \
                     tc.tile_pool(name="pst2", bufs=1, space="PSUM") as ps2, \
                     tc.tile_pool(name="psot", bufs=1, space="PSUM") as pso:
                    stream(gp, sb, ps, ps2, pso, tab_a, W.tidx_a, t_pad,
                           u_pad // 2, True)
                    stream(gp, sb, ps, ps2, pso, tab_b, W.tidx_b, t_pad,
                           u_pad + t_pad // 2, True)
                with tc.tile_pool(name="sbu", bufs=4) as sb, \
                     tc.tile_pool(name="psu", bufs=2, space="PSUM") as ps, \
                     tc.tile_pool(name="psu2", bufs=3, space="PSUM") as ps2, \
                     tc.tile_pool(name="psou", bufs=3, space="PSUM") as pso:
                    stream(gp, sb, ps, ps2, pso, tab_a, W.uidx_a, u_pad,
                           0, False)
                    stream(gp, sb, ps, ps2, pso, tab_b, W.uidx_b, u_pad,
                           (u_pad + t_pad) // 2, False)

    nc.compile()
    return nc


@functools.lru_cache(maxsize=4)
def _cached_program(u_pad, t_pad, ch):
    return build_program(u_pad, t_pad, ch)


def _round_up(x, m):
    return ((x + m - 1) // m) * m


def _prepare(label_nodes, src, dst, t, msg, memory_buf, node_state, last_seen,
             node_features, decay, W_msg, b_msg, W_ih, W_hh, b_ih, b_hh,
             W_feat, b_feat, W1, b1, W2, b2, current_time):
    """Host-side routing/packing. Returns (in_maps, meta)."""
    label_nodes = np.asarray(label_nodes)

    # ---- host: event-level scalars (O(1) work) ----
    t0 = float(np.asarray(t)[0])
    T = float(current_time)
    rdecay = max(float(decay), 0.0)
    event_msg = msg[0].astype(np.float64) @ W_msg.T.astype(np.float64) + b_msg
    gi = event_msg @ W_ih.T.astype(np.float64) + b_ih  # [3H], includes b_ih
    gi = gi.astype(np.float32)
    dec_t = np.float32(np.exp(-rdecay * (T - t0)))

    # ---- host: routing (dedup to unique label nodes) ----
    touched_nodes = np.unique(np.concatenate([src, dst]))
    uniq_vals, inv = np.unique(label_nodes, return_inverse=True)
    is_t = np.isin(uniq_vals, touched_nodes)
    chunk_id = uniq_vals // CH            # 0..15
    local = (uniq_vals % CH).astype(np.int16)

    key = chunk_id.astype(np.int64) * 2 + is_t
    order = np.argsort(key, kind="stable")
    counts = np.bincount(key, minlength=NCHUNK * 2)
    u_counts = counts[0::2]
    t_counts = counts[1::2]
    u_pad = max(_round_up(int(u_counts.max()), G), G)
    t_pad = max(_round_up(int(t_counts.max()), G), G)

    starts = np.zeros(NCHUNK * 2 + 1, dtype=np.int64)
    np.cumsum(counts, out=starts[1:])

    # ---- host: packed 768B-row table ----
    tab8 = np.zeros((N, 2 * ROW), dtype=np.uint8)
    tab8[:, 0:256] = memory_buf.astype(BF16).view(np.uint8)
    tab8[:, 256:512] = node_state.astype(BF16).view(np.uint8)
    blk = np.zeros((N, 256), dtype=np.uint8)
    f8 = (node_features * FSC).astype(FP8).view(np.uint8)
    dec_pre = np.exp(rdecay * last_seen.astype(np.float64)).astype(BF16)
    blk[:, 0:2] = dec_pre.reshape(-1, 1).view(np.uint8)
    blk[:, 2:256:2] = f8[:, 1:128]        # featA: even bytes, units 1..127
    blk[:, 3:91:2] = f8[:, 128:172]       # featB: odd bytes, units 1..44
    blk[:, 91] = f8[:, 0]                 # feat[0]: odd byte, unit 45
    tab8[:, 512:768] = blk
    tab = tab8.view(BF16)                 # [N, 384] bf16-typed bytes

    # ---- host: weights / aux ----
    def bfc(x):
        return np.ascontiguousarray(x, dtype=BF16)

    def f32c(x):
        return np.ascontiguousarray(x, dtype=np.float32).reshape(-1, 1)

    WcT = (W1 @ W_feat).T * FSC  # [F, H]; W_feat folded through W1, x4
    wfab = np.zeros((128, 2, H), np.float32)
    wfab[1:128, 0] = WcT[1:128]           # even-byte plane: feat[1:128]
    wfab[1:45, 1] = WcT[128:172]          # odd-byte plane: feat[128:172]
    wfab[45, 1] = WcT[0]                  # feat[0] at odd byte of unit 45
    b2v2 = np.zeros(OC, np.float32)
    b2v2[0:C] = b2
    b2v2[64:64 + C] = b2
    aux = {
        "wfab": np.ascontiguousarray(wfab, dtype=FP8),
        "w1t": bfc(W1.T * (FSC * FSC)),
        "w2t": bfc(np.concatenate(
            [W2.T, np.zeros((H, 64 - C), np.float32)], axis=1)),
        "whhrt": bfc(W_hh[0:128].T),
        "whhzt": bfc(W_hh[128:256].T),
        "whhnt": bfc(W_hh[256:384].T),
        "ones1": np.ones((1, 128), dtype=BF16),
        "b1p": f32c(b1 + W1 @ b_feat),
        "b2v2": f32c(b2v2),
        "c_r": f32c(gi[0:128] + b_hh[0:128]),
        "c_z": f32c(gi[128:256] + b_hh[128:256]),
        "gin": f32c(gi[256:384]),
        "bhn": f32c(b_hh[256:384]),
        "dect": np.full((128, 1), dec_t, dtype=np.float32),
        "dtc": np.full((128, 1), np.exp(-rdecay * T), dtype=np.float32),
    }

    # ---- host: per-core input maps ----
    in_maps = []
    group_uids = {}  # (chunk, touched) -> unique-label ids in device order
    for ci in range(NCHUNK):
        for tf in (0, 1):
            k = ci * 2 + tf
            group_uids[(ci, tf)] = order[starts[k]:starts[k + 1]]

    def idx_input(ci, tf, pad):
        uids = group_uids[(ci, tf)]
        li = np.zeros(pad, dtype=np.int16)
        li[:uids.shape[0]] = local[uids]
        return _wrap_idxs(li)

    for core in range(NCORES):
        ca, cb = 2 * core, 2 * core + 1
        im = dict(aux)
        im["tab_a"] = tab[ca * CH:(ca + 1) * CH]
        im["tab_b"] = tab[cb * CH:(cb + 1) * CH]
        im["uidx_a"] = idx_input(ca, 0, u_pad)
        im["uidx_b"] = idx_input(cb, 0, u_pad)
        im["tidx_a"] = idx_input(ca, 1, t_pad)
        im["tidx_b"] = idx_input(cb, 1, t_pad)
        in_maps.append(im)

    # column (within a core's virtual [C, totcol] output) of each unique
    # label node; _finish first unpacks the [114, totcol/2] device layout.
    totcol = 2 * (u_pad + t_pad)
    col_of_uniq = np.empty(uniq_vals.shape[0], dtype=np.int64)
    for ci in range(NCHUNK):
        core = ci // 2
        for tf in (0, 1):
            uids = group_uids[(ci, tf)]
            if (ci % 2) == 0:
                c0 = 0 if tf == 0 else u_pad
            else:
                c0 = (u_pad + t_pad) if tf == 0 else (2 * u_pad + t_pad)
            col_of_uniq[uids] = core * totcol + c0 + np.arange(uids.shape[0])

    meta = {"u_pad": u_pad, "t_pad": t_pad, "col_of_uniq": col_of_uniq,
            "inv": inv, "nb": label_nodes.shape[0]}
    return in_maps, meta


def _finish(core_outs, meta):
    """Unpack [114, tch] col-tiled outputs and map back to label order."""
    outs50 = []
    for o in core_outs:
        o = np.asarray(o, dtype=np.float32)
        nh = o.shape[1]
        arr = o.reshape(OC, nh // S, S)
        st = np.stack([arr[0:C], arr[64:64 + C]], axis=2)  # [C, nh/S, 2, S]
        outs50.append(st.reshape(C, 2 * nh))
    combined = np.concatenate(outs50, axis=1)  # [C, NCORES*totcol]
    return np.ascontiguousarray(
        combined[:, meta["col_of_uniq"][meta["inv"]]].T)


def kernel(**inputs):
    inputs = {k: np.asarray(v) for k, v in inputs.items()}
    in_maps, meta = _prepare(**inputs)
    nc = _cached_program(meta["u_pad"], meta["t_pad"], CH)
    res = run_bass_kernel_spmd(nc, in_maps, core_ids=list(range(NCORES)))
    return _finish([r["out"] for r in res.results], meta)


# revision 40
# speedup vs baseline: 1.8300x; 1.0178x over previous
"""DyRep classifier Bass kernel for 8 Trainium2 NeuronCores.

Strategy (self-contained; shapes hardcoded for the target problem):
  - The output depends only on per-label-node rows of (memory_buf,
    node_state[post-event], last_seen[post-event], node_features).
  - Host packs the four per-node tables into one 768-byte row per node
    (vs 1024B for the bf16-only layout):
      block0 (256B): memory_buf bf16 — DVE operand.
      block1 (256B): node_state bf16 — GRU matmul rhs / DVE operand.
      block2 (256B): dec_pre=exp(relu(decay)*last_seen) bf16 at u16-unit
        0, feat*4 as fp8 pairs on units 1..127 (even bytes feat[1:128],
        odd bytes feat[128:172] on units 1..44 and feat[0] on unit 45).
    dec_pre is in [1,1.65) so BOTH its bytes decode as finite fp8 — the
    feat matmuls span it with zero weight rows without NaN risk. The
    feat matmuls run fp8 x fp8 (weights also *4; HW runs mixed
    bf16 x fp8 matmuls incorrectly, and fp8 weights at natural scale
    would be subnormal). The combined x16 is undone for free by the
    ACT scale on the h1 relu; W1 is pre-multiplied by 16 to match.
    No exp on device: dec = dec_pre * exp(-rd*T) via an ACT scale on a
    [1,S] slice, broadcast across partitions by a K=1 ones-matmul.
    Sharded row-wise into 16 chunks of 31250 rows (int16-indexable);
    each of the 8 cores owns 2 chunks.
  - Host routes each unique label node to its owner (core, chunk) and
    splits into "untouched" / "touched" (touched = node hit by the
    event batch, needs the GRU update applied on the fly).
  - Device per core: dma_gather(transpose=True) delivers rows
    feature-major. h1 psum accumulates 3 matmuls (featA fp8, featB fp8,
    W1@(mem + state*dec)), ACT relu with fused bias+scale, then W2
    (zero-padded to 64 rows) col-tiled so both supertiles of a gather
    land in ONE [128, 448] psum bank (s0 -> partitions 0:64,
    s1 -> 64:128), one DVE bias-copy, bf16 half-width output.
  - Host unpermutes the per-core outputs back to label order.
"""

import functools
import os
import numpy as np
import ml_dtypes

import concourse.bass as bass
import concourse.mybir as mybir
import concourse.tile as tile
from concourse import bacc
from concourse.bass_utils import run_bass_kernel_spmd

BF16 = ml_dtypes.bfloat16
FP8 = ml_dtypes.float8_e4m3

# Problem dims (fixed by the task)
N = 500000
H = 128
F = 172
C = 50
B = 200000

NCORES = 8
NCHUNK = 16                  # index chunks (int16 addressing limit)
CH = N // NCHUNK             # 31250 rows per chunk
ROW = 512                    # bf16-unit elements per packed row (1024 bytes)
G = int(os.environ.get("K_G", "896"))   # occurrences per gather
# (transpose dma_gather num_idxs is HW-capped at 1016: 896 largest %128)
S = G // 2                   # supertile: occurrences per compute slice
FSC = 4.0                    # fp8 scale for feat AND its weights (x16 total)
OC = 114                     # packed output partitions (2 x C, col-tiled)
NSWQ = int(os.environ.get("K_NSWQ", "4"))  # SWDGE queues: alternate gathers
# across queues so one gather's ring drain overlaps the next gather's
# descriptor generation (each queue has its own 256-desc ring).

f32 = mybir.dt.float32
bf16 = mybir.dt.bfloat16
fp8 = mybir.dt.float8e4
i16 = mybir.dt.int16
AF = mybir.ActivationFunctionType
OP = mybir.AluOpType


def _wrap_idxs(idx: np.ndarray) -> np.ndarray:
    """Wrap a flat int16 index list into the [128, n/16] SWDGE layout:
    element j at [j%16, j//16], replicated into all 8 16-partition groups."""
    n = idx.shape[0]
    assert n % 16 == 0
    cols = n // 16
    t = np.empty((128, cols), dtype=np.int16)
    blk = idx.reshape(cols, 16).T  # [16, cols]
    for k in range(8):
        t[k * 16:(k + 1) * 16, :] = blk
    return t


def build_program(u_pad: int, t_pad: int, ch: int = CH):
    """Build + compile the SPMD Bass program. Cached by padded sizes."""
    nc = bacc.Bacc("TRN2", target_bir_lowering=False, debug=False,
                   num_devices=NCORES, num_swdge_queues=NSWQ)

    dt_in = {}

    def din(name, shape, dt):
        dt_in[name] = nc.dram_tensor(name, shape, dt, kind="ExternalInput").ap()
        return dt_in[name]

    tab_a = din("tab_a", (ch, ROW), bf16)
    tab_b = din("tab_b", (ch, ROW), bf16)
    uidx_a = din("uidx_a", (128, u_pad // 16), i16)
    uidx_b = din("uidx_b", (128, u_pad // 16), i16)
    tidx_a = din("tidx_a", (128, t_pad // 16), i16)
    tidx_b = din("tidx_b", (128, t_pad // 16), i16)

    wfab = din("wfab", (128, 2, 128), fp8)   # DoubleRow planes: [k,0,:] even
    #   byte weights (feat[k], row0 zero), [k,1,:] odd byte weights
    #   (feat[127+k] for k=1..44, feat[0] at k=45, zero elsewhere); all *4
    w1t = din("w1t", (128, 128), bf16)       # W1.T * 16
    w2t = din("w2t", (128, 64), bf16)        # [W2.T | zeros] (64 cols)
    whhrt = din("whhrt", (128, 128), bf16)   # W_hh[0:128].T
    whhzt = din("whhzt", (128, 128), bf16)   # W_hh[128:256].T
    whhnt = din("whhnt", (128, 128), bf16)   # W_hh[256:384].T
    ones1 = din("ones1", (1, 128), bf16)
    b1p = din("b1p", (128, 1), f32)          # b1 + W1 @ b_feat
    b2v2 = din("b2v2", (OC, 1), f32)         # [b2; 0*14; b2]
    c_r = din("c_r", (128, 1), f32)          # gi_r + b_hh_r
    c_z = din("c_z", (128, 1), f32)          # gi_z + b_hh_z
    gin = din("gin", (128, 1), f32)          # gi_n
    bhn = din("bhn", (128, 1), f32)          # b_hh_n
    dect = din("dect", (128, 1), f32)        # exp(-relu(decay)*(T-t0))
    dtc = din("dtc", (128, 1), f32)          # exp(-relu(decay)*T)

    tch = (u_pad + t_pad)                    # out columns per core (half)
    out = nc.dram_tensor("out", (OC, tch), bf16, kind="ExternalOutput").ap()

    class W:
        pass

    with tile.TileContext(nc) as tc:
        with tc.tile_pool(name="wpool", bufs=1) as wp:
            for name in ("tidx_a", "tidx_b", "uidx_a", "uidx_b",
                         "wfab", "w1t", "w2t", "whhrt", "whhzt",
                         "whhnt", "ones1", "b1p", "b2v2", "c_r", "c_z", "gin",
                         "bhn", "dect", "dtc"):
                ap = dt_in[name]
                t = wp.tile(list(ap.shape), ap.dtype, tag=name)
                nc.sync.dma_start(t[:], ap[:])
                setattr(W, name, t)

            def fab_mm(p_h1, X8, s):
                """Open the h1 psum group: featA+featB in ONE DoubleRow
                matmul consuming the raw fp8 byte pairs of block2."""
                rhs = X8[:, 2, bass.ds(2 * s * S, 2 * S)].rearrange(
                    "p (n two) -> p two n", two=2)
                nc.tensor.matmul(p_h1[:], lhsT=W.wfab[:], rhs=rhs,
                                 start=True, stop=False,
                                 perf_mode=mybir.MatmulPerfMode.DoubleRow)

            def finish_tile(sb, ps2, p_o, X, s, p_h1, sstate):
                """Close the h1 group (W1 matmul), relu, W2 into the
                col-tiled p_o half."""
                sl = bass.ds(s * S, S)
                memT = X[:, 0, sl]
                t1 = sb.tile([128, S], bf16, tag="t1")
                nc.vector.tensor_tensor(out=t1[:], in0=sstate[:], in1=memT,
                                        op=OP.add)
                nc.tensor.matmul(p_h1[:], lhsT=W.w1t[:], rhs=t1[:],
                                 start=False, stop=True)
                h1 = sb.tile([128, S], bf16, tag="h1s")
                nc.scalar.activation(h1[:], p_h1[:], AF.Relu, bias=W.b1p[:],
                                     scale=1.0 / 16.0)
                o_sl = p_o[0:64, :] if s == 0 else p_o[64:128, :]
                nc.tensor.matmul(o_sl, lhsT=W.w2t[:], rhs=h1[:],
                                 start=True, stop=True)

            def untouched_gather(sb, ps, ps2, p_o, X):
                """Both supertiles of one gather, phase-split so the
                gather-ready work (feat DoubleRow matmuls, decayed-state
                scaling off the gather-broadcast dec block) issues before
                the dependent W1/W2 chain."""
                X8 = X[:].bitcast(fp8)
                p_h1, sstates = [], []
                for s in range(2):
                    ph = ps2.tile([128, S], f32, tag="h1")
                    fab_mm(ph, X8, s)
                    p_h1.append(ph)
                    sl = bass.ds(s * S, S)
                    sstate = sb.tile([128, S], bf16, tag="sstate")
                    nc.vector.scalar_tensor_tensor(
                        out=sstate[:], in0=X[:, 1, sl], scalar=W.dtc[:, 0:1],
                        in1=X[:, 3, sl], op0=OP.mult, op1=OP.mult)
                    sstates.append(sstate)
                for s in range(2):
                    finish_tile(sb, ps2, p_o, X, s, p_h1[s], sstates[s])

            def touched_supertile(sb, ps, ps2, p_o, X, s):
                sl = bass.ds(s * S, S)
                X8 = X[:].bitcast(fp8)
                stT = X[:, 1, sl]
                if True:
                    p_r = ps.tile([128, S], f32, tag="gr")
                    nc.tensor.matmul(p_r[:], lhsT=W.whhrt[:], rhs=stT,
                                     start=True, stop=True)
                    p_z = ps.tile([128, S], f32, tag="gz")
                    nc.tensor.matmul(p_z[:], lhsT=W.whhzt[:], rhs=stT,
                                     start=True, stop=True)
                    p_n = ps.tile([128, S], f32, tag="gn")
                    nc.tensor.matmul(p_n[:], lhsT=W.whhnt[:], rhs=stT,
                                     start=True, stop=True)
                    r = sb.tile([128, S], f32, tag="r")
                    nc.scalar.activation(r[:], p_r[:], AF.Sigmoid, bias=W.c_r[:])
                    z = sb.tile([128, S], f32, tag="z")
                    nc.scalar.activation(z[:], p_z[:], AF.Sigmoid, bias=W.c_z[:])
                    hn = sb.tile([128, S], f32, tag="hn")
                    nc.scalar.activation(hn[:], p_n[:], AF.Identity, bias=W.bhn[:])
                    rn = sb.tile([128, S], f32, tag="rn")
                    nc.vector.tensor_tensor(out=rn[:], in0=r[:], in1=hn[:],
                                            op=OP.mult)
                    n = sb.tile([128, S], f32, tag="n")
                    nc.scalar.activation(n[:], rn[:], AF.Tanh, bias=W.gin[:])
                    d = sb.tile([128, S], f32, tag="d")
                    nc.vector.tensor_tensor(out=d[:], in0=stT, in1=n[:],
                                            op=OP.subtract)
                    zd = sb.tile([128, S], f32, tag="zd")
                    nc.vector.tensor_tensor(out=zd[:], in0=z[:], in1=d[:],
                                            op=OP.mult)
                    ns = sb.tile([128, S], f32, tag="ns")
                    nc.vector.tensor_tensor(out=ns[:], in0=n[:], in1=zd[:],
                                            op=OP.add)
                    sstate = sb.tile([128, S], bf16, tag="sstate")
                    nc.vector.tensor_scalar_mul(sstate[:], ns[:],
                                                W.dect[:, 0:1])

                p_h1 = ps2.tile([128, S], f32, tag="h1")
                fab_mm(p_h1, X8, s)
                finish_tile(sb, ps2, p_o, X, s, p_h1, sstate)

            gctr = [0]

            def stream(gp, sb, ps, ps2, pso, table_ap, idx_tile, n_occ,
                       col0h, touched):
                """Process one (chunk, touched?) stream of n_occ occurrences
                in gathers of G; outputs to out[:, col0h : col0h+n_occ/2]."""
                n_g = n_occ // G
                for g in range(n_g):
                    X = gp.tile([128, 4, G], bf16, tag=f"gath{touched}")
                    nc.gpsimd.dma_gather(
                        out_ap=X[:],
                        in_ap=table_ap[:],
                        idxs_ap=idx_tile[:, bass.ds(g * G // 16, G // 16)],
                        num_idxs=G,
                        num_idxs_reg=G,
                        elem_size=ROW,
                        transpose=True,
                        queue_num=gctr[0] % NSWQ,
                    )
                    gctr[0] += 1
                    p_o = pso.tile([128, S], f32, tag=f"po{touched}")
                    if touched:
                        for s in range(2):
                            touched_supertile(sb, ps, ps2, p_o, X, s)
                    else:
                        untouched_gather(sb, ps, ps2, p_o, X)
                    osb = gp.tile([OC, S], bf16, tag=f"osb{touched}")
                    nc.vector.tensor_scalar_add(osb[:], p_o[0:OC, :],
                                                W.b2v2[:, 0:1])
                    nc.sync.dma_start(
                        out[:, bass.ds(col0h + g * S, S)], osb[:])

            # One deep gather pool shared by both phases so the Pool engine
            # streams descriptor generation continuously across the
            # touched->untouched transition. Touched first: its serial GRU
            # chain drains while untouched gathers already issue.
            with tc.tile_pool(name="gp", bufs=8) as gp:
                with tc.tile_pool(name="sbt", bufs=2) as sb, \
                     tc.tile_pool(name="pst", bufs=1, space="PSUM") as ps, \
                     tc.tile_pool(name="pst2", bufs=1, space="PSUM") as ps2, \
                     tc.tile_pool(name="psot", bufs=1, space="PSUM") as pso:
                    stream(gp, sb, ps, ps2, pso, tab_a, W.tidx_a, t_pad,
                           u_pad // 2, True)
                    stream(gp, sb, ps, ps2, pso, tab_b, W.tidx_b, t_pad,
                           u_pad + t_pad // 2, True)
                with tc.tile_pool(name="sbu", bufs=4) as sb, \
                     tc.tile_pool(name="psu", bufs=1, space="PSUM") as ps, \
                     tc.tile_pool(name="psu2", bufs=4, space="PSUM") as ps2, \
                     tc.tile_pool(name="psou", bufs=4, space="PSUM") as pso:
                    stream(gp, sb, ps, ps2, pso, tab_a, W.uidx_a, u_pad,
                           0, False)
                    stream(gp, sb, ps, ps2, pso, tab_b, W.uidx_b, u_pad,
                           (u_pad + t_pad) // 2, False)

    nc.compile()
    return nc


@functools.lru_cache(maxsize=4)
def _cached_program(u_pad, t_pad, ch):
    return build_program(u_pad, t_pad, ch)


def _round_up(x, m):
    return ((x + m - 1) // m) * m


def _prepare(label_nodes, src, dst, t, msg, memory_buf, node_state, last_seen,
             node_features, decay, W_msg, b_msg, W_ih, W_hh, b_ih, b_hh,
             W_feat, b_feat, W1, b1, W2, b2, current_time):
    """Host-side routing/packing. Returns (in_maps, meta)."""
    label_nodes = np.asarray(label_nodes)

    # ---- host: event-level scalars (O(1) work) ----
    t0 = float(np.asarray(t)[0])
    T = float(current_time)
    rdecay = max(float(decay), 0.0)
    event_msg = msg[0].astype(np.float64) @ W_msg.T.astype(np.float64) + b_msg
    gi = event_msg @ W_ih.T.astype(np.float64) + b_ih  # [3H], includes b_ih
    gi = gi.astype(np.float32)
    dec_t = np.float32(np.exp(-rdecay * (T - t0)))

    # ---- host: routing (dedup to unique label nodes) ----
    touched_nodes = np.unique(np.concatenate([src, dst]))
    uniq_vals, inv = np.unique(label_nodes, return_inverse=True)
    is_t = np.isin(uniq_vals, touched_nodes)
    chunk_id = uniq_vals // CH            # 0..15
    local = (uniq_vals % CH).astype(np.int16)

    key = chunk_id.astype(np.int64) * 2 + is_t
    order = np.argsort(key, kind="stable")
    counts = np.bincount(key, minlength=NCHUNK * 2)
    u_counts = counts[0::2]
    t_counts = counts[1::2]
    u_pad = max(_round_up(int(u_counts.max()), G), G)
    t_pad = max(_round_up(int(t_counts.max()), G), G)

    starts = np.zeros(NCHUNK * 2 + 1, dtype=np.int64)
    np.cumsum(counts, out=starts[1:])

    # ---- host: packed 768B-row table ----
    tab8 = np.zeros((N, 2 * ROW), dtype=np.uint8)
    tab8[:, 0:256] = memory_buf.astype(BF16).view(np.uint8)
    tab8[:, 256:512] = node_state.astype(BF16).view(np.uint8)
    blk = np.zeros((N, 256), dtype=np.uint8)
    f8 = (node_features * FSC).astype(FP8).view(np.uint8)
    dec_pre = np.exp(rdecay * last_seen.astype(np.float64)).astype(BF16)
    blk[:, 0:2] = dec_pre.reshape(-1, 1).view(np.uint8)
    blk[:, 2:256:2] = f8[:, 1:128]        # featA: even bytes, units 1..127
    blk[:, 3:91:2] = f8[:, 128:172]       # featB: odd bytes, units 1..44
    blk[:, 91] = f8[:, 0]                 # feat[0]: odd byte, unit 45
    tab8[:, 512:768] = blk
    # block3: dec_pre replicated into all 128 units -> the transpose
    # gather broadcasts it across partitions for free.
    tab8[:, 768:1024] = np.broadcast_to(
        dec_pre.reshape(-1, 1, 1).view(np.uint8), (N, 128, 2)).reshape(N, 256)
    tab = tab8.view(BF16)                 # [N, 512] bf16-typed bytes

    # ---- host: weights / aux ----
    def bfc(x):
        return np.ascontiguousarray(x, dtype=BF16)

    def f32c(x):
        return np.ascontiguousarray(x, dtype=np.float32).reshape(-1, 1)

    WcT = (W1 @ W_feat).T * FSC  # [F, H]; W_feat folded through W1, x4
    wfab = np.zeros((128, 2, H), np.float32)
    wfab[1:128, 0] = WcT[1:128]           # even-byte plane: feat[1:128]
    wfab[1:45, 1] = WcT[128:172]          # odd-byte plane: feat[128:172]
    wfab[45, 1] = WcT[0]                  # feat[0] at odd byte of unit 45
    b2v2 = np.zeros(OC, np.float32)
    b2v2[0:C] = b2
    b2v2[64:64 + C] = b2
    aux = {
        "wfab": np.ascontiguousarray(wfab, dtype=FP8),
        "w1t": bfc(W1.T * (FSC * FSC)),
        "w2t": bfc(np.concatenate(
            [W2.T, np.zeros((H, 64 - C), np.float32)], axis=1)),
        "whhrt": bfc(W_hh[0:128].T),
        "whhzt": bfc(W_hh[128:256].T),
        "whhnt": bfc(W_hh[256:384].T),
        "ones1": np.ones((1, 128), dtype=BF16),
        "b1p": f32c(b1 + W1 @ b_feat),
        "b2v2": f32c(b2v2),
        "c_r": f32c(gi[0:128] + b_hh[0:128]),
        "c_z": f32c(gi[128:256] + b_hh[128:256]),
        "gin": f32c(gi[256:384]),
        "bhn": f32c(b_hh[256:384]),
        "dect": np.full((128, 1), dec_t, dtype=np.float32),
        "dtc": np.full((128, 1), np.exp(-rdecay * T), dtype=np.float32),
    }

    # ---- host: per-core input maps ----
    in_maps = []
    group_uids = {}  # (chunk, touched) -> unique-label ids in device order
    for ci in range(NCHUNK):
        for tf in (0, 1):
            k = ci * 2 + tf
            group_uids[(ci, tf)] = order[starts[k]:starts[k + 1]]

    def idx_input(ci, tf, pad):
        uids = group_uids[(ci, tf)]
        li = np.zeros(pad, dtype=np.int16)
        li[:uids.shape[0]] = local[uids]
        return _wrap_idxs(li)

    for core in range(NCORES):
        ca, cb = 2 * core, 2 * core + 1
        im = dict(aux)
        im["tab_a"] = tab[ca * CH:(ca + 1) * CH]
        im["tab_b"] = tab[cb * CH:(cb + 1) * CH]
        im["uidx_a"] = idx_input(ca, 0, u_pad)
        im["uidx_b"] = idx_input(cb, 0, u_pad)
        im["tidx_a"] = idx_input(ca, 1, t_pad)
        im["tidx_b"] = idx_input(cb, 1, t_pad)
        in_maps.append(im)

    # column (within a core's virtual [C, totcol] output) of each unique
    # label node; _finish first unpacks the [114, totcol/2] device layout.
    totcol = 2 * (u_pad + t_pad)
    col_of_uniq = np.empty(uniq_vals.shape[0], dtype=np.int64)
    for ci in range(NCHUNK):
        core = ci // 2
        for tf in (0, 1):
            uids = group_uids[(ci, tf)]
            if (ci % 2) == 0:
                c0 = 0 if tf == 0 else u_pad
            else:
                c0 = (u_pad + t_pad) if tf == 0 else (2 * u_pad + t_pad)
            col_of_uniq[uids] = core * totcol + c0 + np.arange(uids.shape[0])

    meta = {"u_pad": u_pad, "t_pad": t_pad, "col_of_uniq": col_of_uniq,
            "inv": inv, "nb": label_nodes.shape[0]}
    return in_maps, meta


def _finish(core_outs, meta):
    """Unpack [114, tch] col-tiled outputs and map back to label order."""
    outs50 = []
    for o in core_outs:
        o = np.asarray(o, dtype=np.float32)
        nh = o.shape[1]
        arr = o.reshape(OC, nh // S, S)
        st = np.stack([arr[0:C], arr[64:64 + C]], axis=2)  # [C, nh/S, 2, S]
        outs50.append(st.reshape(C, 2 * nh))
    combined = np.concatenate(outs50, axis=1)  # [C, NCORES*totcol]
    return np.ascontiguousarray(
        combined[:, meta["col_of_uniq"][meta["inv"]]].T)


def kernel(**inputs):
    inputs = {k: np.asarray(v) for k, v in inputs.items()}
    in_maps, meta = _prepare(**inputs)
    nc = _cached_program(meta["u_pad"], meta["t_pad"], CH)
    res = run_bass_kernel_spmd(nc, in_maps, core_ids=list(range(NCORES)))
    return _finish([r["out"] for r in res.results], meta)
